# revision 23
# baseline (speedup 1.0000x reference)
"""GIN message-passing (3 layers + JumpingKnowledge cat + Linear) on 8 TRN2 NeuronCores.

Strategy (graph/data parallel, edges sharded by destination node):
  - Nodes are partitioned into 8 shards of 12544 rows, each split into 2
    half-shards of 6272 (6250 real + 22 pad) so every layer pipelines as
    gather(half1) || mlp(half0) || AllGather(half0).  Core c computes the
    aggregation + MLP for its own dst shard and gathers source features from a
    full local copy of h laid out as [half-plane][core][6272] (per-half
    AllGathers write contiguous slices; the input x itself is AllGathered
    on-device from per-core shards, so no full-size upload).
  - The segment_sum gather uses the GPSIMD dma_gather primitive (int16 indices,
    256B rows, <=1024 descriptors per call: the NRT-programmed SWDGE ring cap).
    Since int16 only addresses 32768 rows, sources are split into 4 windows of
    32768 positions; per (core, half, window) the dst nodes are sorted by
    in-degree and packed into groups of 128 with per-chunk-uniform slot counts,
    padding slots pointing at an all-zero (pad) row of h.
  - Gather index tables are uploaded de-replicated ([16, n]) and expanded to
    the ucode's 128-partition layout on-device once at kernel start.
  - Gathered tiles [128 nodes, D slots, 64ch] are segment-summed on the Vector
    engine (strided reduce over the slot axis), assembled per (half, window) in
    token (degree-sorted) order, stored to DRAM, and gathered back per node
    during the MLP phase (unique indices -> no RMW collisions).
  - MLP runs channel-major on the TensorEngine (transpose via PE identity
    matmul), biases+ReLU on the Scalar engine.  The JumpingKnowledge concat +
    final Linear is folded into the layer loop: each layer's channel-major
    activation is immediately matmul'd with its lin_W block and accumulated
    into an SBUF-resident bf16 accumulator, so layer-2 activations never touch
    DRAM and no separate JK phase re-reads the shards.
"""

import os
import sys

os.environ.setdefault("MYCRO_LOCAL_CACHE", "1")
if "/opt/trn_rl_repo" not in sys.path:
    sys.path.insert(0, "/opt/trn_rl_repo")

from contextlib import ExitStack
from dataclasses import dataclass, field

import numpy as np


# --------------------------------------------------------------------------- #
# configuration
# --------------------------------------------------------------------------- #
@dataclass
class Cfg:
    n: int = 100000          # real nodes
    e: int = 1600000
    c: int = 64              # channels (in == hid == 64)
    ncores: int = 8
    nhalves: int = 2         # half-shards per core (pipeline granularity)
    cols_max: int = 48       # gather-chunk column budget (slots per partition)
    tile_n: int = 512        # MLP node-tile width
    n_layers: int = 3
    nqueues: int = 4         # SWDGE queues; round-robin unlocks SDMA parallelism
    # NRT programs the SWDGE ring carveout at a fixed 1024 descriptors; a
    # single gather call must fit (larger calls hang in ucode await_space).
    scratch: int = 16384     # SWDGE descriptor-ring carveout (bytes/partition)
    qcols: int = 8           # gather-call column cap (128*qcols idxs <= ring)
    bn: int = 1024           # nodes per agg-gather-back call (<= ring)
    # layer-0 source: "full" uploads x replicated (free on the re-dispatch
    # path); "allgather" builds it on-device (less upload, +1 collective)
    x_input: str = "full"

    hreal: int = field(init=False)   # real nodes per half-shard
    hsz: int = field(init=False)     # rows per half-shard (multiple of 128)
    hgroups: int = field(init=False)
    nsh: int = field(init=False)     # rows per core shard
    nv: int = field(init=False)      # total virtual half-shards
    ntot: int = field(init=False)
    hp: int = field(init=False)      # rows per half-plane of hf
    window: int = field(init=False)  # gather window (never crosses a plane)
    nwin: int = field(init=False)

    def __post_init__(self):
        self.qcols = int(os.environ.get("K_QCOLS", self.qcols))
        self.bn = int(os.environ.get("K_BN", self.bn))
        self.x_input = os.environ.get("K_XINPUT", self.x_input)
        self.nhalves = int(os.environ.get("K_NH", self.nhalves))
        self.cols_max = int(os.environ.get("K_COLSMAX", self.cols_max))
        assert self.n % (self.ncores * self.nhalves) == 0
        self.hreal = self.n // (self.ncores * self.nhalves)
        self.hgroups = -(-self.hreal // 128)
        self.hsz = self.hgroups * 128
        self.nsh = self.hsz * self.nhalves
        self.nv = self.ncores * self.nhalves
        self.ntot = self.hsz * self.nv
        self.hp = self.hsz * self.ncores
        # split each half-plane into equal windows of <= 32768 rows (int16)
        wsplit = -(-self.hp // 32768)
        assert self.hp % wsplit == 0
        self.window = self.hp // wsplit
        self.nwin = self.ntot // self.window
        assert self.window % self.hsz == 0, "windows must contain whole vshards"
        assert self.hsz % 16 == 0
        assert self.hsz > self.hreal, "need pad rows to host the zero rows"
        assert self.tile_n % 128 == 0
        assert self.bn % 16 == 0


# hf row of source node s: [half-plane][core][pos-in-half]
def _src_pos(cfg: Cfg, s):
    c = s // (cfg.hreal * cfg.nhalves)
    r = s % (cfg.hreal * cfg.nhalves)
    h = r // cfg.hreal
    j = r % cfg.hreal
    return h * (cfg.ncores * cfg.hsz) + c * cfg.hsz + j


@dataclass
class Sched:
    """Cross-core-uniform gather schedule, per (half, window).

    chunks[h][w] = list of (g0, [D_g ...], padc) — consecutive active groups
    packed into one SBUF tile; gather calls split at qcols columns.
    """
    chunks: list
    gmax: list
    totc: list    # [h][w] total idx columns (sum nidx/16)
    zr: list      # [w] absolute hf position of an all-zero (pad) row


# --------------------------------------------------------------------------- #
# host-side preprocessing
# --------------------------------------------------------------------------- #
def _wrap_idx(v: np.ndarray) -> np.ndarray:
    """int16 vector (len % 16 == 0) -> [16, len/16] wrapped layout.

    Logical index j lives at [j % 16, j // 16]; on-device the 16-partition
    pattern is replicated to all 128 partitions (ucode cores each read their
    own stripe).
    """
    return np.ascontiguousarray(v.reshape(-1, 16).T)  # [16, len/16]


def preprocess(edge_index: np.ndarray, cfg: Cfg):
    nc_, nh, nw, hsz = cfg.ncores, cfg.nhalves, cfg.nwin, cfg.hsz
    src = edge_index[0].astype(np.int64)
    dst = edge_index[1].astype(np.int64)

    ps = _src_pos(cfg, src)                       # hf position of each source
    w_of_src = ps // cfg.window

    c_of = dst // (cfg.hreal * nh)
    rd = dst % (cfg.hreal * nh)
    h_of = rd // cfg.hreal
    ld = rd % cfg.hreal                           # dst pos within half

    # key: (core, half, window, local-dst)
    key = (((c_of * nh + h_of) * nw) + w_of_src) * hsz + ld
    order = np.lexsort((ps, key))
    key_s = key[order]
    ps_s = ps[order]
    cnt = np.bincount(key_s, minlength=nc_ * nh * nw * hsz).reshape(
        nc_, nh, nw, hsz
    )
    base = np.zeros(nc_ * nh * nw * hsz + 1, np.int64)
    base[1:] = np.cumsum(cnt.ravel())

    # window row counts + zero rows (pad rows stay zero every layer)
    wrows = [min(cfg.window, cfg.ntot - w * cfg.window) for w in range(nw)]
    pad_pos = np.concatenate(
        [
            np.arange(v * hsz + cfg.hreal, (v + 1) * hsz)
            for v in range(cfg.nv)
        ]
    )
    zr = []
    for w in range(nw):
        cand = pad_pos[(pad_pos >= w * cfg.window) & (pad_pos < w * cfg.window + wrows[w])]
        assert cand.size, f"window {w} has no zero row"
        zr.append(int(cand[0]))

    # per-(core,half,window) degree-sorted node order
    pi = np.empty((nc_, nh, nw, hsz), np.int64)
    for c in range(nc_):
        for h in range(nh):
            for w in range(nw):
                pi[c, h, w] = np.argsort(-cnt[c, h, w], kind="stable")

    # shared schedule: per-group D = cross-core max of group max degree
    chunks = [[None] * nw for _ in range(nh)]
    gmax = [[0] * nw for _ in range(nh)]
    totc = [[0] * nw for _ in range(nh)]
    for h in range(nh):
        for w in range(nw):
            sd = -np.sort(-cnt[:, h, w, :], axis=1)
            D = sd[:, ::128].max(axis=0)           # [hgroups]
            gm = int(np.count_nonzero(D))
            ch, g0 = [], 0
            while g0 < gm:
                ds, g1 = [], g0
                while g1 < gm and sum(ds) + D[g1] <= cfg.cols_max:
                    ds.append(int(D[g1]))
                    g1 += 1
                if not ds:
                    ds = [int(D[g0])]
                    g1 = g0 + 1
                padc = (-sum(ds)) % 8
                ch.append((g0, ds, padc))
                g0 = g1
            chunks[h][w] = ch
            gmax[h][w] = gm
            totc[h][w] = sum(128 * (sum(ds) + padc) // 16 for (_, ds, padc) in ch)

    # per-core gather / scatter index arrays ([16, n] de-replicated layout)
    gidx = [[[None] * nw for _ in range(nh)] for _ in range(nc_)]
    sidx = [[[None] * nw for _ in range(nh)] for _ in range(nc_)]
    tot_slots = 0
    for c in range(nc_):
        for h in range(nh):
            for w in range(nw):
                parts = []
                zr_loc = zr[w] - w * cfg.window
                for (g0, ds, padc) in chunks[h][w]:
                    cols = sum(ds)
                    nidx = 128 * cols
                    col2g = np.repeat(np.arange(len(ds)), ds)
                    col2d = np.concatenate([np.arange(dd) for dd in ds])
                    i = np.arange(nidx)
                    p = i & 127
                    col = i >> 7
                    r = (g0 + col2g[col]) * 128 + p
                    d = col2d[col]
                    node = pi[c, h, w][r]
                    deg = cnt[c, h, w, node]
                    k = ((c * nh + h) * nw + w) * hsz + node
                    e = base[k] + np.minimum(d, np.maximum(deg - 1, 0))
                    e = np.minimum(e, max(len(ps_s) - 1, 0))
                    v = ps_s[e] - w * cfg.window
                    v = np.where(d < deg, v, zr_loc)
                    if padc:
                        v = np.concatenate(
                            [v, np.full(128 * padc, zr_loc, np.int64)]
                        )
                    assert v.min() >= 0 and v.max() < wrows[w]
                    parts.append(_wrap_idx(v.astype(np.int16)))
                    tot_slots += nidx + 128 * padc
                gidx[c][h][w] = np.concatenate(parts, axis=1)
                rank = np.empty(hsz, np.int64)
                rank[pi[c, h, w]] = np.arange(hsz)
                sidx[c][h][w] = _wrap_idx(rank.astype(np.int16))

    pad_frac = tot_slots / max(len(src), 1)
    return Sched(chunks, gmax, totc, zr), gidx, sidx, wrows, pad_frac


# --------------------------------------------------------------------------- #
# device program
# --------------------------------------------------------------------------- #
def _patch_queue_affine_sem_lanes():
    """Make Tile's DMASW lane assignment queue-affine.

    SWDGE completion sems are locked to the queue that first increments them
    (ucode sem_target is per-queue).  Tile cycles lanes 0..7 in scheduled
    order, which breaks once calls use queue_num 1..3.  Map queue q to lanes
    {q, q+4} so every lane only ever serves one queue.
    """
    import concourse.tile_sem_assignment as tsa
    import concourse.bass_isa as bass_isa
    import concourse.mybir as mybir

    if getattr(tsa, "_queue_affine_patched", False):
        return
    orig = tsa.TileClockTick._assign_tick
    DMAInst = tsa.DMAInst

    def _assign_tick(self, inst):
        if (
            inst.engine == mybir.EngineType.Pool
            and isinstance(inst, DMAInst)
            and not isinstance(inst, bass_isa.UserSyncedRemoteDMADescs)
            and self.swdge_sem_count == 8
        ):
            q = int(getattr(inst, "queue_num", 0) or 0) % 4
            tog = getattr(self, "_q_lane_toggle", None)
            if tog is None:
                tog = self._q_lane_toggle = {}
            t = tog.get(q, 0)
            tog[q] = t ^ 1
            self.next_sw_dma_idx = q + 4 * t
        return orig(self, inst)

    tsa.TileClockTick._assign_tick = _assign_tick
    tsa._queue_affine_patched = True


def build_program(cfg: Cfg, sched: Sched, wrows, debug=None, reps=1):
    import concourse.bacc as bacc
    import concourse.mybir as mybir
    import concourse.tile as tile
    from concourse.masks import make_identity

    if cfg.nqueues > 1:
        _patch_queue_affine_sem_lanes()

    f32 = mybir.dt.float32
    bf16 = mybir.dt.bfloat16
    i16 = mybir.dt.int16
    C = cfg.c
    P = 128
    NL = cfg.n_layers
    NH = cfg.nhalves
    HP = cfg.ncores * cfg.hsz    # rows per half-plane of hf

    nc = bacc.Bacc(
        "TRN2",
        target_bir_lowering=False,
        debug=False,
        num_devices=cfg.ncores,
        num_swdge_queues=cfg.nqueues,
        dynamic_dma_scratch_size=cfg.scratch,
    )
    qrr = iter(range(10**9))  # round-robin SWDGE queue counter

    def nextq():
        return next(qrr) % cfg.nqueues

    x_shard = nc.dram_tensor("x_shard", [cfg.nsh, C], f32, kind="ExternalInput")
    x_full = (
        nc.dram_tensor("x_full", [cfg.ntot, C], f32, kind="ExternalInput")
        if cfg.x_input == "full"
        else None
    )
    gidx_t = [
        [
            nc.dram_tensor(
                f"gidx_h{h}w{w}", [16, sched.totc[h][w]], i16, kind="ExternalInput"
            )
            for w in range(cfg.nwin)
        ]
        for h in range(NH)
    ]
    sidx_t = [
        [
            nc.dram_tensor(
                f"sidx_h{h}w{w}", [16, cfg.hsz // 16], i16, kind="ExternalInput"
            )
            for w in range(cfg.nwin)
        ]
        for h in range(NH)
    ]
    w1_t, b1_t, w2_t, b2_t = [], [], [], []
    for l in range(NL):
        w1_t.append(nc.dram_tensor(f"W1_{l}", [C, C], f32, kind="ExternalInput"))
        b1_t.append(nc.dram_tensor(f"b1_{l}", [C, 1], f32, kind="ExternalInput"))
        w2_t.append(nc.dram_tensor(f"W2_{l}", [C, C], f32, kind="ExternalInput"))
        b2_t.append(nc.dram_tensor(f"b2_{l}", [C, 1], f32, kind="ExternalInput"))
    linw_t = nc.dram_tensor("lin_W", [NL * C, C], f32, kind="ExternalInput")
    linb_t = nc.dram_tensor("lin_b", [C, 1], f32, kind="ExternalInput")
    out_t = nc.dram_tensor("out_shard", [cfg.nsh, C], f32, kind="ExternalOutput")

    rg = [list(range(cfg.ncores))]
    add = mybir.AluOpType.add
    relu = mybir.ActivationFunctionType.Relu

    with tile.TileContext(nc) as tc, ExitStack() as ctx:
        const = ctx.enter_context(tc.tile_pool(name="const", bufs=1))
        accp = ctx.enter_context(tc.tile_pool(name="accp", bufs=1))
        idxp = ctx.enter_context(tc.tile_pool(name="idx", bufs=3))
        gat = ctx.enter_context(tc.tile_pool(name="gat", bufs=3))
        asmp = ctx.enter_context(tc.tile_pool(name="asm", bufs=2))
        mlp = ctx.enter_context(tc.tile_pool(name="mlp", bufs=2))
        psum = ctx.enter_context(tc.tile_pool(name="psum", bufs=2, space="PSUM"))
        dram = ctx.enter_context(tc.tile_pool(name="dram", bufs=1, space="DRAM"))

        # ---- constants ----
        identity = const.tile([P, P], f32)
        make_identity(nc, identity[:])
        ZCH = 16  # zero-fill chunk (groups per DMA)
        zeros = const.tile([P, ZCH * C], f32)
        nc.vector.memset(zeros[:], 0.0)
        w1s, b1s, w2s, b2s, lws = [], [], [], [], []
        for l in range(NL):
            w1 = const.tile([C, C], f32, name=f"w1s_{l}")
            nc.sync.dma_start(out=w1[:], in_=w1_t[l].ap())
            w1s.append(w1)
            b1 = const.tile([C, 1], f32, name=f"b1s_{l}")
            nc.sync.dma_start(out=b1[:], in_=b1_t[l].ap())
            b1s.append(b1)
            w2 = const.tile([C, C], f32, name=f"w2s_{l}")
            nc.sync.dma_start(out=w2[:], in_=w2_t[l].ap())
            w2s.append(w2)
            b2 = const.tile([C, 1], f32, name=f"b2s_{l}")
            nc.sync.dma_start(out=b2[:], in_=b2_t[l].ap())
            b2s.append(b2)
            lw = const.tile([C, C], f32, name=f"lws_{l}")
            nc.sync.dma_start(out=lw[:], in_=linw_t.ap()[l * C : (l + 1) * C, :])
            lws.append(lw)
        lb = const.tile([C, 1], f32)
        nc.sync.dma_start(out=lb[:], in_=linb_t.ap())
        sis = [[None] * cfg.nwin for _ in range(NH)]
        for h in range(NH):
            for w in range(cfg.nwin):
                si = const.tile([P, cfg.hsz // 16], i16, name=f"sis_{h}_{w}")
                for k in range(8):
                    nc.sync.dma_start(
                        out=si[16 * k : 16 * (k + 1), :], in_=sidx_t[h][w].ap()
                    )
                sis[h][w] = si

        # ---- internal DRAM ----
        # ucode-layout gather indices, expanded 16 -> 128 partitions once
        gidx_rep = [
            [
                dram.tile([P, sched.totc[h][w]], i16, name=f"gidxr_{h}_{w}")
                for w in range(cfg.nwin)
            ]
            for h in range(NH)
        ]
        for h in range(NH):
            for w in range(cfg.nwin):
                for k in range(8):
                    nc.sync.dma_start(
                        out=gidx_rep[h][w][16 * k : 16 * (k + 1), :],
                        in_=gidx_t[h][w].ap(),
                    )
        # collectives can't read IO tensors: stage x_shard into internal DRAM
        xs_int = None
        if cfg.x_input != "full":
            xs_int = dram.tile([cfg.nsh, C], f32, name="xs_int")
            nc.sync.dma_start(out=xs_int[:], in_=x_shard.ap())
        # per-(half,window) partial aggregations, in token order
        bufw = [
            [
                dram.tile([cfg.hsz, C], f32, name=f"bufw_{h}_{w}")
                for w in range(cfg.nwin)
            ]
            for h in range(NH)
        ]
        shard = [dram.tile([cfg.nsh, C], f32, name=f"shard_{l}") for l in range(NL - 1)]
        # Shared tensors allow a single writer only -> one tensor per
        # (rep, layer, half-plane); each per-half AllGather owns one.
        # hf[l][h] = AllGather of half h (layer-l output, or x for l=0).
        hf_all = [
            [
                None
                if (l == 0 and cfg.x_input == "full")
                else [
                    dram.tile(
                        [HP, C], f32, name=f"hf_{r}_{l}_{h}", addr_space="Shared"
                    )
                    for h in range(NH)
                ]
                for l in range(NL)
            ]
            for r in range(reps)
        ]

        # JumpingKnowledge accumulator: sum_l h_l @ lin_W_l, channel-major
        acc = accp.tile([C, cfg.nsh], bf16, name="jkacc")

        npad = cfg.hsz - cfg.hreal
        cc_full = cfg.tile_n // 128

        def transpose_in(src_ap, dst_ap, cc):
            """node-major [128, cc*C] -> channel-major [C, cc*128]."""
            for s in range(cc):
                pt = psum.tile([C, P], f32, name="tp", tag="tp")
                nc.tensor.transpose(
                    out=pt[:], in_=src_ap[:, s * C : (s + 1) * C], identity=identity[:]
                )
                nc.scalar.copy(out=dst_ap[:, s * P : (s + 1) * P], in_=pt[:])

        def transpose_out(src_ap, dst_ap, cc):
            """channel-major [C, cc*128] -> node-major [128, cc*C]."""
            for s in range(cc):
                pt = psum.tile([P, C], f32, name="tpo", tag="tp")
                nc.tensor.transpose(
                    out=pt[:],
                    in_=src_ap[:, s * P : (s + 1) * P],
                    identity=identity[:C, :C],
                )
                nc.scalar.copy(out=dst_ap[:, s * C : (s + 1) * C], in_=pt[:])

        def all_gather(src_ap, dst_ap):
            if "fakecc" in (debug or ""):
                # timeline-sim mode: stand in for the AllGather with a
                # local DMA of similar cost (TimelineSim can't do CC)
                nc.sync.dma_start(out=dst_ap[: cfg.hsz, :], in_=src_ap)
            else:
                nc.gpsimd.collective_compute(
                    "AllGather",
                    mybir.AluOpType.bypass,
                    replica_groups=rg,
                    ins=[src_ap],
                    outs=[dst_ap],
                )

        # (reps>1 repeats the whole 3-layer pipeline for slope-based timing;
        # results are idempotent since rep>0 re-reads the same x inputs)
        for _rep, l in ((r, ll) for r in range(reps) for ll in range(NL)):
            hf = hf_all[_rep]
            if l == 0:
                nc.vector.memset(acc[:], 0.0)
                if cfg.x_input != "full":
                    for h in range(NH):
                        all_gather(
                            xs_int[h * cfg.hsz : (h + 1) * cfg.hsz],
                            hf[0][h][:],
                        )

            wpp = HP // cfg.window    # windows per half-plane

            def win_src(w):
                r0 = (w % wpp) * cfg.window
                if l == 0 and cfg.x_input == "full":
                    return x_full.ap()[w * cfg.window : (w + 1) * cfg.window, :]
                return hf[l][w // wpp][r0 : r0 + cfg.window, :]

            hcur = x_shard.ap() if l == 0 else shard[l - 1][:]

            for hh in range(NH):
                # ---- gather + segment-sum, per source window ----
                for w in range(0 if "mlponly" in (debug or "") else cfg.nwin):
                    win = win_src(w)
                    asm = asmp.tile([P, cfg.hgroups * C], f32, name="asm", tag="asm")
                    ioff = 0
                    for (g0, ds, padc) in sched.chunks[hh][w]:
                        cols = sum(ds) + padc
                        it = idxp.tile([P, cols * 8], i16, name="it", tag="it")
                        nc.sync.dma_start(
                            out=it[:], in_=gidx_rep[hh][w][:, ioff : ioff + cols * 8]
                        )
                        ioff += cols * 8
                        T = gat.tile([P, cols * C], f32, name="gt", tag="gt")
                        co = 0
                        while co < cols:
                            nco = min(cfg.qcols, cols - co)
                            nc.gpsimd.dma_gather(
                                out_ap=T[:, co * C : (co + nco) * C].rearrange(
                                    "p (k f) -> p k f", f=C
                                ),
                                in_ap=win,
                                idxs_ap=it[:, co * 8 : (co + nco) * 8],
                                num_idxs=128 * nco,
                                num_idxs_reg=128 * nco,
                                elem_size=C,
                                queue_num=nextq(),
                            )
                            co += nco
                        # segment-sum each group's slots (strided reduce)
                        goff = 0
                        for gi, dd in enumerate(ds):
                            nc.vector.tensor_reduce(
                                out=asm[:, (g0 + gi) * C : (g0 + gi + 1) * C],
                                in_=T[:, goff * C : (goff + dd) * C].rearrange(
                                    "p (d f) -> p f d", f=C
                                ),
                                axis=mybir.AxisListType.X,
                                op=add,
                            )
                            goff += dd
                    if sched.gmax[hh][w] < cfg.hgroups:
                        nc.vector.memset(asm[:, sched.gmax[hh][w] * C :], 0.0)
                    # store token-ordered partials; the MLP phase gathers them
                    # back by node (no serialized scatter-add chain needed)
                    nc.sync.dma_start(
                        out=bufw[hh][w][:].rearrange("(k p) f -> p k f", p=P),
                        in_=asm[:].rearrange("p (k f) -> p k f", f=C),
                    )

                # ---- m = h + Σ_w agg_w (gather-back) ; MLP ; JK accum ----
                BN = cfg.bn
                b0 = 0
                while b0 < cfg.hsz:
                    bn = min(BN, cfg.hsz - b0)
                    Gs = []
                    if "mlponly" not in (debug or ""):
                        for w in range(cfg.nwin):
                            G = mlp.tile(
                                [P, (BN // 128) * C], f32, name=f"G{w}", tag=f"G{w}"
                            )
                            nc.gpsimd.dma_gather(
                                out_ap=G[:, : (bn // 128) * C].rearrange(
                                    "p (k f) -> p k f", f=C
                                ),
                                in_ap=bufw[hh][w][:],
                                idxs_ap=sis[hh][w][:, b0 // 16 : (b0 + bn) // 16],
                                num_idxs=bn,
                                num_idxs_reg=bn,
                                elem_size=C,
                                queue_num=nextq(),
                            )
                            Gs.append(G)
                    for s0 in range(0, bn, cfg.tile_n):
                        t0 = hh * cfg.hsz + b0 + s0     # shard-relative row
                        tn = min(cfg.tile_n, bn - s0)
                        cc = tn // 128
                        ksl = slice((s0 // 128) * C, (s0 // 128 + cc) * C)
                        A = mlp.tile([P, cc_full * C], f32, name="A", tag="A")
                        H = mlp.tile([P, cc_full * C], f32, name="H", tag="H")
                        nc.sync.dma_start(
                            out=H[:, : cc * C].rearrange("p (k f) -> p k f", f=C),
                            in_=hcur[t0 : t0 + tn, :].rearrange(
                                "(k p) f -> p k f", p=P
                            ),
                        )
                        if Gs:
                            nc.vector.tensor_tensor(
                                out=A[:, : cc * C], in0=Gs[0][:, ksl],
                                in1=Gs[1][:, ksl], op=add,
                            )
                            for G in Gs[2:]:
                                nc.vector.tensor_tensor(
                                    out=A[:, : cc * C], in0=A[:, : cc * C],
                                    in1=G[:, ksl], op=add,
                                )
                            nc.vector.tensor_tensor(
                                out=A[:, : cc * C], in0=A[:, : cc * C],
                                in1=H[:, : cc * C], op=add,
                            )
                        else:
                            nc.vector.tensor_copy(
                                out=A[:, : cc * C], in_=H[:, : cc * C]
                            )
                        if "aggonly" in (debug or ""):
                            nc.sync.dma_start(
                                out=out_t.ap()[t0 : t0 + tn, :].rearrange(
                                    "(k p) f -> p k f", p=P
                                ),
                                in_=A[:, : cc * C].rearrange("p (k f) -> p k f", f=C),
                            )
                            continue
                        mT = mlp.tile([C, cfg.tile_n], f32, name="mT", tag="mT")
                        transpose_in(A[:], mT[:], cc)
                        Y = psum.tile([C, cfg.tile_n], f32, name="Y", tag="Y")
                        nc.tensor.matmul(
                            out=Y[:, :tn], lhsT=w1s[l][:], rhs=mT[:, :tn],
                            start=True, stop=True,
                        )
                        Ys = mlp.tile([C, cfg.tile_n], f32, name="Ys", tag="Ys")
                        nc.scalar.activation(
                            out=Ys[:, :tn], in_=Y[:, :tn], func=relu, bias=b1s[l][:]
                        )
                        Z = psum.tile([C, cfg.tile_n], f32, name="Z", tag="Y")
                        nc.tensor.matmul(
                            out=Z[:, :tn], lhsT=w2s[l][:], rhs=Ys[:, :tn],
                            start=True, stop=True,
                        )
                        Hn = mlp.tile([C, cfg.tile_n], f32, name="Hn", tag="Hn")
                        nc.scalar.activation(
                            out=Hn[:, :tn], in_=Z[:, :tn], func=relu, bias=b2s[l][:]
                        )
                        # JK fold: acc[:, tile] += h_{l+1} @ lin_W_l
                        jk = psum.tile([C, cfg.tile_n], f32, name="jk", tag="jk")
                        nc.tensor.matmul(
                            out=jk[:, :tn], lhsT=lws[l][:], rhs=Hn[:, :tn],
                            start=True, stop=True,
                        )
                        nc.vector.tensor_tensor(
                            out=acc[:, t0 : t0 + tn], in0=acc[:, t0 : t0 + tn],
                            in1=jk[:, :tn], op=add,
                        )
                        if l < NL - 1:
                            Hm = mlp.tile([P, cc_full * C], f32, name="Hm", tag="Hm")
                            transpose_out(Hn[:], Hm[:], cc)
                            nc.sync.dma_start(
                                out=shard[l][t0 : t0 + tn, :].rearrange(
                                    "(k p) f -> p k f", p=P
                                ),
                                in_=Hm[:, : cc * C].rearrange(
                                    "p (k f) -> p k f", f=C
                                ),
                            )
                    b0 += bn

                # zero this half's pad rows, then replicate to every core
                if l < NL - 1 and "aggonly" not in (debug or ""):
                    h0 = hh * cfg.hsz
                    nc.sync.dma_start(
                        out=shard[l][h0 + cfg.hreal : h0 + cfg.hsz, :],
                        in_=zeros[:npad, :C],
                    )
                    all_gather(shard[l][h0 : h0 + cfg.hsz], hf[l + 1][hh][:])
            if "aggonly" in (debug or ""):
                break

        # ---- JK bias + ReLU + store (per node tile) ----
        if "aggonly" not in (debug or ""):
            t0 = 0
            while t0 < cfg.nsh:
                tn = min(cfg.tile_n, cfg.nsh - t0)
                cc = tn // 128
                O = mlp.tile([C, cfg.tile_n], f32, name="O", tag="Hn")
                nc.scalar.activation(
                    out=O[:, :tn], in_=acc[:, t0 : t0 + tn], func=relu, bias=lb[:]
                )
                Om = mlp.tile([P, cc_full * C], f32, name="Om", tag="Hm")
                transpose_out(O[:], Om[:], cc)
                nc.sync.dma_start(
                    out=out_t.ap()[t0 : t0 + tn, :].rearrange(
                        "(k p) f -> p k f", p=P
                    ),
                    in_=Om[:, : cc * C].rearrange("p (k f) -> p k f", f=C),
                )
                t0 += tn

    nc.compile()
    return nc


# --------------------------------------------------------------------------- #
# host orchestration
# --------------------------------------------------------------------------- #
def make_in_maps(cfg: Cfg, gidx, sidx, x, weights):
    HP = cfg.ncores * cfg.hsz
    xf = None
    if cfg.x_input == "full":
        xf = np.zeros((cfg.ntot, cfg.c), np.float32)
        for c in range(cfg.ncores):
            for h in range(cfg.nhalves):
                r0 = (c * cfg.nhalves + h) * cfg.hreal
                xf[h * HP + c * cfg.hsz : h * HP + c * cfg.hsz + cfg.hreal] = x[
                    r0 : r0 + cfg.hreal
                ]
    in_maps = []
    for c in range(cfg.ncores):
        xs = np.zeros((cfg.nsh, cfg.c), np.float32)
        for h in range(cfg.nhalves):
            r0 = (c * cfg.nhalves + h) * cfg.hreal
            xs[h * cfg.hsz : h * cfg.hsz + cfg.hreal] = x[r0 : r0 + cfg.hreal]
        m = {"x_shard": xs}
        if xf is not None:
            m["x_full"] = xf
        for h in range(cfg.nhalves):
            for w in range(cfg.nwin):
                m[f"gidx_h{h}w{w}"] = gidx[c][h][w]
                m[f"sidx_h{h}w{w}"] = sidx[c][h][w]
        for l in range(cfg.n_layers):
            m[f"W1_{l}"] = weights[f"W1_{l}"]
            m[f"b1_{l}"] = weights[f"b1_{l}"].reshape(cfg.c, 1)
            m[f"W2_{l}"] = weights[f"W2_{l}"]
            m[f"b2_{l}"] = weights[f"b2_{l}"].reshape(cfg.c, 1)
        m["lin_W"] = weights["lin_W"]
        m["lin_b"] = weights["lin_b"].reshape(cfg.c, 1)
        in_maps.append(m)
    return in_maps


def assemble_output(cfg: Cfg, results):
    out = np.empty((cfg.n, cfg.c), np.float32)
    for c in range(cfg.ncores):
        for h in range(cfg.nhalves):
            r0 = (c * cfg.nhalves + h) * cfg.hreal
            out[r0 : r0 + cfg.hreal] = results[c]["out_shard"][
                h * cfg.hsz : h * cfg.hsz + cfg.hreal
            ]
    return out


def run_on_hw(nc, in_maps, cfg: Cfg, trace=False):
    from concourse.bass_utils import run_bass_kernel_spmd

    res = run_bass_kernel_spmd(
        nc, in_maps, core_ids=list(range(cfg.ncores)), trace=trace
    )
    return res


def kernel(**inputs) -> np.ndarray:
    x = np.asarray(inputs["x"], np.float32)
    edge_index = np.asarray(inputs["edge_index"])
    cfg = Cfg()
    assert x.shape == (cfg.n, cfg.c)
    sched, gidx, sidx, wrows, pad = preprocess(edge_index, cfg)
    nc = build_program(cfg, sched, wrows)
    in_maps = make_in_maps(cfg, gidx, sidx, x, inputs)
    res = run_on_hw(nc, in_maps, cfg)
    return assemble_output(cfg, res.results)


# revision 33
# speedup vs baseline: 1.3888x; 1.3888x over previous
"""GIN message-passing (3 layers + JumpingKnowledge cat + Linear) on 8 TRN2 NeuronCores.

Strategy (graph/data parallel, edges sharded by destination node):
  - Nodes are partitioned into 8 shards of 12544 rows, each split into 2
    half-shards of 6272 (6250 real + 22 pad) so every layer pipelines as
    gather(half1) || mlp(half0) || AllGather(half0).  Core c computes the
    aggregation + MLP for its own dst shard and gathers source features from a
    full local copy of h laid out as [half-plane][core][6272] (per-half
    AllGathers write contiguous slices; the input x itself is AllGathered
    on-device from per-core shards, so no full-size upload).
  - The segment_sum gather uses the GPSIMD dma_gather primitive (int16 indices,
    256B rows, <=1024 descriptors per call: the NRT-programmed SWDGE ring cap).
    Since int16 only addresses 32768 rows, sources are split into 4 windows of
    32768 positions; per (core, half, window) the dst nodes are sorted by
    in-degree and packed into groups of 128 with per-chunk-uniform slot counts,
    padding slots pointing at an all-zero (pad) row of h.
  - Gather index tables are uploaded de-replicated ([16, n]) and expanded to
    the ucode's 128-partition layout on-device once at kernel start.
  - Gathered tiles [128 nodes, D slots, 64ch] are segment-summed on the Vector
    engine (strided reduce over the slot axis), assembled per (half, window) in
    token (degree-sorted) order, stored to DRAM, and gathered back per node
    during the MLP phase (unique indices -> no RMW collisions).
  - MLP runs channel-major on the TensorEngine (transpose via PE identity
    matmul), biases+ReLU on the Scalar engine.  The JumpingKnowledge concat +
    final Linear is folded into the layer loop: each layer's channel-major
    activation is immediately matmul'd with its lin_W block and accumulated
    into an SBUF-resident bf16 accumulator, so layer-2 activations never touch
    DRAM and no separate JK phase re-reads the shards.
"""

import os
import sys

os.environ.setdefault("MYCRO_LOCAL_CACHE", "1")
if "/opt/trn_rl_repo" not in sys.path:
    sys.path.insert(0, "/opt/trn_rl_repo")

from contextlib import ExitStack
from dataclasses import dataclass, field

import numpy as np


# --------------------------------------------------------------------------- #
# configuration
# --------------------------------------------------------------------------- #
@dataclass
class Cfg:
    n: int = 100000          # real nodes
    e: int = 1600000
    c: int = 64              # channels (in == hid == 64)
    ncores: int = 8
    nhalves: int = 2         # half-shards per core (pipeline granularity)
    cols_max: int = 48       # gather-chunk column budget (slots per partition)
    tile_n: int = 512        # MLP node-tile width
    n_layers: int = 3
    nqueues: int = 4         # SWDGE queues; round-robin unlocks SDMA parallelism
    # NRT programs the SWDGE ring carveout at a fixed 1024 descriptors; a
    # single gather call must fit (larger calls hang in ucode await_space).
    scratch: int = 16384     # SWDGE descriptor-ring carveout (bytes/partition)
    qcols: int = 8           # gather-call column cap (128*qcols idxs <= ring)
    bn: int = 1024           # nodes per agg-gather-back call (<= ring)
    # layer-0 source: "full" uploads x replicated (free on the re-dispatch
    # path); "allgather" builds it on-device (less upload, +1 collective)
    x_input: str = "full"

    hreal: int = field(init=False)   # real nodes per half-shard
    hsz: int = field(init=False)     # rows per half-shard (multiple of 128)
    hgroups: int = field(init=False)
    nsh: int = field(init=False)     # rows per core shard
    nv: int = field(init=False)      # total virtual half-shards
    ntot: int = field(init=False)
    hp: int = field(init=False)      # rows per half-plane of hf
    window: int = field(init=False)  # gather window (never crosses a plane)
    nwin: int = field(init=False)

    def __post_init__(self):
        self.qcols = int(os.environ.get("K_QCOLS", self.qcols))
        self.bn = int(os.environ.get("K_BN", self.bn))
        self.x_input = os.environ.get("K_XINPUT", self.x_input)
        self.nhalves = int(os.environ.get("K_NH", self.nhalves))
        self.cols_max = int(os.environ.get("K_COLSMAX", self.cols_max))
        assert self.n % (self.ncores * self.nhalves) == 0
        self.hreal = self.n // (self.ncores * self.nhalves)
        self.hgroups = -(-self.hreal // 128)
        self.hsz = self.hgroups * 128
        self.nsh = self.hsz * self.nhalves
        self.nv = self.ncores * self.nhalves
        self.ntot = self.hsz * self.nv
        self.hp = self.hsz * self.ncores
        # split each half-plane into equal windows of <= 32768 rows (int16)
        wsplit = -(-self.hp // 32768)
        assert self.hp % wsplit == 0
        self.window = self.hp // wsplit
        self.nwin = self.ntot // self.window
        assert self.window % self.hsz == 0, "windows must contain whole vshards"
        assert self.hsz % 16 == 0
        assert self.hsz > self.hreal, "need pad rows to host the zero rows"
        assert self.tile_n % 128 == 0
        assert self.bn % 16 == 0


# hf row of source node s: [half-plane][core][pos-in-half]
def _src_pos(cfg: Cfg, s):
    c = s // (cfg.hreal * cfg.nhalves)
    r = s % (cfg.hreal * cfg.nhalves)
    h = r // cfg.hreal
    j = r % cfg.hreal
    return h * (cfg.ncores * cfg.hsz) + c * cfg.hsz + j


@dataclass
class Sched:
    """Cross-core-uniform gather schedule, per (half, window).

    chunks[h][w] = list of (g0, [D_g ...], padc) — consecutive active groups
    packed into one SBUF tile; gather calls split at qcols columns.
    """
    chunks: list
    gmax: list
    totc: list    # [h][w] total idx columns (sum nidx/16)
    zr: list      # [w] absolute hf position of an all-zero (pad) row


# --------------------------------------------------------------------------- #
# host-side preprocessing
# --------------------------------------------------------------------------- #
def _wrap_idx(v: np.ndarray) -> np.ndarray:
    """int16 vector (len % 16 == 0) -> [16, len/16] wrapped layout.

    Logical index j lives at [j % 16, j // 16]; on-device the 16-partition
    pattern is replicated to all 128 partitions (ucode cores each read their
    own stripe).
    """
    return np.ascontiguousarray(v.reshape(-1, 16).T)  # [16, len/16]


def preprocess(edge_index: np.ndarray, cfg: Cfg):
    nc_, nh, nw, hsz = cfg.ncores, cfg.nhalves, cfg.nwin, cfg.hsz
    src = edge_index[0].astype(np.int64)
    dst = edge_index[1].astype(np.int64)

    ps = _src_pos(cfg, src)                       # hf position of each source
    w_of_src = ps // cfg.window

    c_of = dst // (cfg.hreal * nh)
    rd = dst % (cfg.hreal * nh)
    h_of = rd // cfg.hreal
    ld = rd % cfg.hreal                           # dst pos within half

    # key: (core, half, window, local-dst)
    key = (((c_of * nh + h_of) * nw) + w_of_src) * hsz + ld
    order = np.lexsort((ps, key))
    key_s = key[order]
    ps_s = ps[order]
    cnt = np.bincount(key_s, minlength=nc_ * nh * nw * hsz).reshape(
        nc_, nh, nw, hsz
    )
    base = np.zeros(nc_ * nh * nw * hsz + 1, np.int64)
    base[1:] = np.cumsum(cnt.ravel())

    # window row counts + zero rows (pad rows stay zero every layer)
    wrows = [min(cfg.window, cfg.ntot - w * cfg.window) for w in range(nw)]
    pad_pos = np.concatenate(
        [
            np.arange(v * hsz + cfg.hreal, (v + 1) * hsz)
            for v in range(cfg.nv)
        ]
    )
    zr = []
    for w in range(nw):
        cand = pad_pos[(pad_pos >= w * cfg.window) & (pad_pos < w * cfg.window + wrows[w])]
        assert cand.size, f"window {w} has no zero row"
        zr.append(int(cand[0]))

    # per-(core,half,window) degree-sorted node order
    pi = np.empty((nc_, nh, nw, hsz), np.int64)
    for c in range(nc_):
        for h in range(nh):
            for w in range(nw):
                pi[c, h, w] = np.argsort(-cnt[c, h, w], kind="stable")

    # shared schedule: per-group D = cross-core max of group max degree
    chunks = [[None] * nw for _ in range(nh)]
    gmax = [[0] * nw for _ in range(nh)]
    totc = [[0] * nw for _ in range(nh)]
    for h in range(nh):
        for w in range(nw):
            sd = -np.sort(-cnt[:, h, w, :], axis=1)
            D = sd[:, ::128].max(axis=0)           # [hgroups]
            gm = int(np.count_nonzero(D))
            ch, g0 = [], 0
            while g0 < gm:
                ds, g1 = [], g0
                while g1 < gm and sum(ds) + D[g1] <= cfg.cols_max:
                    ds.append(int(D[g1]))
                    g1 += 1
                if not ds:
                    ds = [int(D[g0])]
                    g1 = g0 + 1
                padc = (-sum(ds)) % 8
                ch.append((g0, ds, padc))
                g0 = g1
            chunks[h][w] = ch
            gmax[h][w] = gm
            totc[h][w] = sum(128 * (sum(ds) + padc) // 16 for (_, ds, padc) in ch)

    # per-core gather / scatter index arrays ([16, n] de-replicated layout)
    gidx = [[[None] * nw for _ in range(nh)] for _ in range(nc_)]
    sidx = [[None] * nh for _ in range(nc_)]
    tot_slots = 0
    for c in range(nc_):
        for h in range(nh):
            for w in range(nw):
                parts = []
                zr_loc = zr[w] - w * cfg.window
                for (g0, ds, padc) in chunks[h][w]:
                    cols = sum(ds)
                    nidx = 128 * cols
                    col2g = np.repeat(np.arange(len(ds)), ds)
                    col2d = np.concatenate([np.arange(dd) for dd in ds])
                    i = np.arange(nidx)
                    p = i & 127
                    col = i >> 7
                    r = (g0 + col2g[col]) * 128 + p
                    d = col2d[col]
                    node = pi[c, h, w][r]
                    deg = cnt[c, h, w, node]
                    k = ((c * nh + h) * nw + w) * hsz + node
                    e = base[k] + np.minimum(d, np.maximum(deg - 1, 0))
                    e = np.minimum(e, max(len(ps_s) - 1, 0))
                    v = ps_s[e] - w * cfg.window
                    v = np.where(d < deg, v, zr_loc)
                    if padc:
                        v = np.concatenate(
                            [v, np.full(128 * padc, zr_loc, np.int64)]
                        )
                    assert v.min() >= 0 and v.max() < wrows[w]
                    parts.append(_wrap_idx(v.astype(np.int16)))
                    tot_slots += nidx + 128 * padc
                gidx[c][h][w] = np.concatenate(parts, axis=1)
            # combined gather-back stream: per node, its nw window-partial
            # rows in bufw ([w*hsz + rank_w(node)]), interleaved (col%nw = w)
            assert nw * hsz <= 32768, "combined gather-back needs int16 range"
            rank = np.empty((nw, hsz), np.int64)
            for w in range(nw):
                rank[w][pi[c, h, w]] = np.arange(hsz)
            i = np.arange(nw * hsz)
            j = i >> 7
            p = i & 127
            node = (j // nw) * 128 + p
            w_ = j % nw
            v = w_ * hsz + rank[w_, node]
            sidx[c][h] = _wrap_idx(v.astype(np.int16))

    pad_frac = tot_slots / max(len(src), 1)
    return Sched(chunks, gmax, totc, zr), gidx, sidx, wrows, pad_frac


# --------------------------------------------------------------------------- #
# device program
# --------------------------------------------------------------------------- #
def _patch_queue_affine_sem_lanes():
    """Make Tile's DMASW lane assignment queue-affine.

    SWDGE completion sems are locked to the queue that first increments them
    (ucode sem_target is per-queue).  Tile cycles lanes 0..7 in scheduled
    order, which breaks once calls use queue_num 1..3.  Map queue q to lanes
    {q, q+4} so every lane only ever serves one queue.
    """
    import concourse.tile_sem_assignment as tsa
    import concourse.bass_isa as bass_isa
    import concourse.mybir as mybir

    if getattr(tsa, "_queue_affine_patched", False):
        return
    orig = tsa.TileClockTick._assign_tick
    DMAInst = tsa.DMAInst

    def _assign_tick(self, inst):
        if (
            inst.engine == mybir.EngineType.Pool
            and isinstance(inst, DMAInst)
            and not isinstance(inst, bass_isa.UserSyncedRemoteDMADescs)
            and self.swdge_sem_count == 8
        ):
            q = int(getattr(inst, "queue_num", 0) or 0) % 4
            tog = getattr(self, "_q_lane_toggle", None)
            if tog is None:
                tog = self._q_lane_toggle = {}
            t = tog.get(q, 0)
            tog[q] = t ^ 1
            self.next_sw_dma_idx = q + 4 * t
        return orig(self, inst)

    tsa.TileClockTick._assign_tick = _assign_tick
    tsa._queue_affine_patched = True


def build_program(cfg: Cfg, sched: Sched, wrows, debug=None, reps=1):
    import concourse.bacc as bacc
    import concourse.mybir as mybir
    import concourse.tile as tile
    from concourse.masks import make_identity

    if cfg.nqueues > 1:
        _patch_queue_affine_sem_lanes()

    f32 = mybir.dt.float32
    bf16 = mybir.dt.bfloat16
    i16 = mybir.dt.int16
    C = cfg.c
    P = 128
    NL = cfg.n_layers
    NH = cfg.nhalves
    HP = cfg.ncores * cfg.hsz    # rows per half-plane of hf

    nc = bacc.Bacc(
        "TRN2",
        target_bir_lowering=False,
        debug=False,
        num_devices=cfg.ncores,
        num_swdge_queues=cfg.nqueues,
        dynamic_dma_scratch_size=cfg.scratch,
    )
    qrr = iter(range(10**9))  # round-robin SWDGE queue counter

    def nextq():
        return next(qrr) % cfg.nqueues

    x_shard = nc.dram_tensor("x_shard", [cfg.nsh, C], f32, kind="ExternalInput")
    x_full = (
        nc.dram_tensor("x_full", [cfg.ntot, C], f32, kind="ExternalInput")
        if cfg.x_input == "full"
        else None
    )
    gidx_t = [
        [
            nc.dram_tensor(
                f"gidx_h{h}w{w}", [16, sched.totc[h][w]], i16, kind="ExternalInput"
            )
            for w in range(cfg.nwin)
        ]
        for h in range(NH)
    ]
    sidx_t = [
        nc.dram_tensor(
            f"sidx_h{h}", [16, cfg.nwin * cfg.hsz // 16], i16, kind="ExternalInput"
        )
        for h in range(NH)
    ]
    w1_t, b1_t, w2_t, b2_t = [], [], [], []
    for l in range(NL):
        w1_t.append(nc.dram_tensor(f"W1_{l}", [C, C], f32, kind="ExternalInput"))
        b1_t.append(nc.dram_tensor(f"b1_{l}", [C, 1], f32, kind="ExternalInput"))
        w2_t.append(nc.dram_tensor(f"W2_{l}", [C, C], f32, kind="ExternalInput"))
        b2_t.append(nc.dram_tensor(f"b2_{l}", [C, 1], f32, kind="ExternalInput"))
    linw_t = nc.dram_tensor("lin_W", [NL * C, C], f32, kind="ExternalInput")
    linb_t = nc.dram_tensor("lin_b", [C, 1], f32, kind="ExternalInput")
    out_t = nc.dram_tensor("out_shard", [cfg.nsh, C], f32, kind="ExternalOutput")

    rg = [list(range(cfg.ncores))]
    add = mybir.AluOpType.add
    relu = mybir.ActivationFunctionType.Relu

    with tile.TileContext(nc) as tc, ExitStack() as ctx:
        const = ctx.enter_context(tc.tile_pool(name="const", bufs=1))
        accp = ctx.enter_context(tc.tile_pool(name="accp", bufs=1))
        idxp = ctx.enter_context(tc.tile_pool(name="idx", bufs=3))
        gat = ctx.enter_context(tc.tile_pool(name="gat", bufs=3))
        asmp = ctx.enter_context(tc.tile_pool(name="asm", bufs=2))
        mlp = ctx.enter_context(tc.tile_pool(name="mlp", bufs=2))
        psum = ctx.enter_context(tc.tile_pool(name="psum", bufs=2, space="PSUM"))
        dram = ctx.enter_context(tc.tile_pool(name="dram", bufs=1, space="DRAM"))

        # ---- constants ----
        identity = const.tile([P, P], f32)
        make_identity(nc, identity[:])
        ZCH = 16  # zero-fill chunk (groups per DMA)
        zeros = const.tile([P, ZCH * C], f32)
        nc.vector.memset(zeros[:], 0.0)
        w1s, b1s, w2s, b2s, lws = [], [], [], [], []
        for l in range(NL):
            w1 = const.tile([C, C], f32, name=f"w1s_{l}")
            nc.sync.dma_start(out=w1[:], in_=w1_t[l].ap())
            w1s.append(w1)
            b1 = const.tile([C, 1], f32, name=f"b1s_{l}")
            nc.sync.dma_start(out=b1[:], in_=b1_t[l].ap())
            b1s.append(b1)
            w2 = const.tile([C, C], f32, name=f"w2s_{l}")
            nc.sync.dma_start(out=w2[:], in_=w2_t[l].ap())
            w2s.append(w2)
            b2 = const.tile([C, 1], f32, name=f"b2s_{l}")
            nc.sync.dma_start(out=b2[:], in_=b2_t[l].ap())
            b2s.append(b2)
            lw = const.tile([C, C], f32, name=f"lws_{l}")
            nc.sync.dma_start(out=lw[:], in_=linw_t.ap()[l * C : (l + 1) * C, :])
            lws.append(lw)
        lb = const.tile([C, 1], f32)
        nc.sync.dma_start(out=lb[:], in_=linb_t.ap())
        sis = []
        for h in range(NH):
            si = const.tile(
                [P, cfg.nwin * cfg.hsz // 16], i16, name=f"sis_{h}"
            )
            for k in range(8):
                nc.sync.dma_start(
                    out=si[16 * k : 16 * (k + 1), :], in_=sidx_t[h].ap()
                )
            sis.append(si)

        # ---- internal DRAM ----
        # ucode-layout gather indices, expanded 16 -> 128 partitions once
        gidx_rep = [
            [
                dram.tile([P, sched.totc[h][w]], i16, name=f"gidxr_{h}_{w}")
                for w in range(cfg.nwin)
            ]
            for h in range(NH)
        ]
        for h in range(NH):
            for w in range(cfg.nwin):
                for k in range(8):
                    nc.sync.dma_start(
                        out=gidx_rep[h][w][16 * k : 16 * (k + 1), :],
                        in_=gidx_t[h][w].ap(),
                    )
        # collectives can't read IO tensors: stage x_shard into internal DRAM
        xs_int = None
        if cfg.x_input != "full":
            xs_int = dram.tile([cfg.nsh, C], f32, name="xs_int")
            nc.sync.dma_start(out=xs_int[:], in_=x_shard.ap())
        # per-half partial aggregations: window w's token-ordered partials at
        # rows [w*hsz, (w+1)*hsz) (one tensor so one gather-back stream reads
        # all windows' entries for a node in a single call family)
        bufw = [
            dram.tile([cfg.nwin * cfg.hsz, C], f32, name=f"bufw_{h}")
            for h in range(NH)
        ]
        shard = [dram.tile([cfg.nsh, C], f32, name=f"shard_{l}") for l in range(NL - 1)]
        # Shared tensors allow a single writer only -> one tensor per
        # (rep, layer, half-plane); each per-half AllGather owns one.
        # hf[l][h] = AllGather of half h (layer-l output, or x for l=0).
        hf_all = [
            [
                None
                if (l == 0 and cfg.x_input == "full")
                else [
                    dram.tile(
                        [HP, C], f32, name=f"hf_{r}_{l}_{h}", addr_space="Shared"
                    )
                    for h in range(NH)
                ]
                for l in range(NL)
            ]
            for r in range(reps)
        ]

        # JumpingKnowledge accumulator: sum_l h_l @ lin_W_l, channel-major
        acc = accp.tile([C, cfg.nsh], bf16, name="jkacc")

        npad = cfg.hsz - cfg.hreal
        cc_full = cfg.tile_n // 128

        def transpose_in(src_ap, dst_ap, cc):
            """node-major [128, cc*C] -> channel-major [C, cc*128]."""
            for s in range(cc):
                pt = psum.tile([C, P], f32, name="tp", tag="tp")
                nc.tensor.transpose(
                    out=pt[:], in_=src_ap[:, s * C : (s + 1) * C], identity=identity[:]
                )
                nc.scalar.copy(out=dst_ap[:, s * P : (s + 1) * P], in_=pt[:])

        def transpose_out(src_ap, dst_ap, cc):
            """channel-major [C, cc*128] -> node-major [128, cc*C]."""
            for s in range(cc):
                pt = psum.tile([P, C], f32, name="tpo", tag="tp")
                nc.tensor.transpose(
                    out=pt[:],
                    in_=src_ap[:, s * P : (s + 1) * P],
                    identity=identity[:C, :C],
                )
                nc.scalar.copy(out=dst_ap[:, s * C : (s + 1) * C], in_=pt[:])

        def all_gather(src_ap, dst_ap):
            if "fakecc" in (debug or ""):
                # timeline-sim mode: stand in for the AllGather with a
                # local DMA of similar cost (TimelineSim can't do CC)
                nc.sync.dma_start(out=dst_ap[: cfg.hsz, :], in_=src_ap)
            else:
                nc.gpsimd.collective_compute(
                    "AllGather",
                    mybir.AluOpType.bypass,
                    replica_groups=rg,
                    ins=[src_ap],
                    outs=[dst_ap],
                )

        # (reps>1 repeats the whole 3-layer pipeline for slope-based timing;
        # results are idempotent since rep>0 re-reads the same x inputs)
        for _rep, l in ((r, ll) for r in range(reps) for ll in range(NL)):
            hf = hf_all[_rep]
            if l == 0:
                nc.vector.memset(acc[:], 0.0)
                if cfg.x_input != "full":
                    for h in range(NH):
                        all_gather(
                            xs_int[h * cfg.hsz : (h + 1) * cfg.hsz],
                            hf[0][h][:],
                        )

            wpp = HP // cfg.window    # windows per half-plane

            def win_src(w):
                r0 = (w % wpp) * cfg.window
                if l == 0 and cfg.x_input == "full":
                    return x_full.ap()[w * cfg.window : (w + 1) * cfg.window, :]
                return hf[l][w // wpp][r0 : r0 + cfg.window, :]

            hcur = x_shard.ap() if l == 0 else shard[l - 1][:]

            for hh in range(NH):
                # ---- gather + segment-sum, per source window ----
                for w in range(0 if "mlponly" in (debug or "") else cfg.nwin):
                    win = win_src(w)
                    asm = asmp.tile([P, cfg.hgroups * C], f32, name="asm", tag="asm")
                    ioff = 0
                    for (g0, ds, padc) in sched.chunks[hh][w]:
                        cols = sum(ds) + padc
                        it = idxp.tile([P, cols * 8], i16, name="it", tag="it")
                        nc.sync.dma_start(
                            out=it[:], in_=gidx_rep[hh][w][:, ioff : ioff + cols * 8]
                        )
                        ioff += cols * 8
                        T = gat.tile([P, cols * C], f32, name="gt", tag="gt")
                        co = 0
                        while co < cols:
                            nco = min(cfg.qcols, cols - co)
                            nc.gpsimd.dma_gather(
                                out_ap=T[:, co * C : (co + nco) * C].rearrange(
                                    "p (k f) -> p k f", f=C
                                ),
                                in_ap=win,
                                idxs_ap=it[:, co * 8 : (co + nco) * 8],
                                num_idxs=128 * nco,
                                num_idxs_reg=128 * nco,
                                elem_size=C,
                                queue_num=nextq(),
                            )
                            co += nco
                        # segment-sum each group's slots (strided reduce)
                        goff = 0
                        for gi, dd in enumerate(ds):
                            nc.vector.tensor_reduce(
                                out=asm[:, (g0 + gi) * C : (g0 + gi + 1) * C],
                                in_=T[:, goff * C : (goff + dd) * C].rearrange(
                                    "p (d f) -> p f d", f=C
                                ),
                                axis=mybir.AxisListType.X,
                                op=add,
                            )
                            goff += dd
                    if sched.gmax[hh][w] < cfg.hgroups:
                        nc.vector.memset(asm[:, sched.gmax[hh][w] * C :], 0.0)
                    # store token-ordered partials; the MLP phase gathers them
                    # back by node (no serialized scatter-add chain needed)
                    nc.sync.dma_start(
                        out=bufw[hh][w * cfg.hsz : (w + 1) * cfg.hsz].rearrange(
                            "(k p) f -> p k f", p=P
                        ),
                        in_=asm[:].rearrange("p (k f) -> p k f", f=C),
                    )

                # ---- m = h + Σ_w agg_w (combined gather-back) ; MLP ; JK ----
                NW = cfg.nwin
                s0 = 0
                while s0 < cfg.hsz:
                    t0 = hh * cfg.hsz + s0     # shard-relative row
                    tn = min(cfg.tile_n, cfg.hsz - s0)
                    cc = tn // 128
                    if True:
                        A = mlp.tile([P, cc_full * C], f32, name="A", tag="A")
                        H = mlp.tile([P, cc_full * C], f32, name="H", tag="H")
                        nc.sync.dma_start(
                            out=H[:, : cc * C].rearrange("p (k f) -> p k f", f=C),
                            in_=hcur[t0 : t0 + tn, :].rearrange(
                                "(k p) f -> p k f", p=P
                            ),
                        )
                        if "mlponly" not in (debug or ""):
                            G = mlp.tile(
                                [P, cc_full * NW * C], f32, name="G", tag="G"
                            )
                            ni = NW * tn
                            io = 0
                            while io < ni:
                                nn_ = min(1024, ni - io)
                                nc.gpsimd.dma_gather(
                                    out_ap=G[
                                        :, (io // 128) * C : ((io + nn_) // 128) * C
                                    ].rearrange("p (k f) -> p k f", f=C),
                                    in_ap=bufw[hh][:],
                                    idxs_ap=sis[hh][
                                        :,
                                        (NW * s0 + io) // 16 : (NW * s0 + io + nn_)
                                        // 16,
                                    ],
                                    num_idxs=nn_,
                                    num_idxs_reg=nn_,
                                    elem_size=C,
                                    queue_num=nextq(),
                                )
                                io += nn_
                            # A[p, q, f] = Σ_w G[p, (q w) f]
                            nc.vector.tensor_reduce(
                                out=A[:, : cc * C].rearrange(
                                    "p (q f) -> p q f", f=C
                                ),
                                in_=G[:, : cc * NW * C].rearrange(
                                    "p (q w f) -> p q f w", w=NW, f=C
                                ),
                                axis=mybir.AxisListType.X,
                                op=add,
                            )
                            nc.vector.tensor_tensor(
                                out=A[:, : cc * C], in0=A[:, : cc * C],
                                in1=H[:, : cc * C], op=add,
                            )
                        else:
                            nc.vector.tensor_copy(
                                out=A[:, : cc * C], in_=H[:, : cc * C]
                            )
                        if "aggonly" in (debug or ""):
                            nc.sync.dma_start(
                                out=out_t.ap()[t0 : t0 + tn, :].rearrange(
                                    "(k p) f -> p k f", p=P
                                ),
                                in_=A[:, : cc * C].rearrange("p (k f) -> p k f", f=C),
                            )
                            s0 += tn
                            continue
                        mT = mlp.tile([C, cfg.tile_n], f32, name="mT", tag="mT")
                        transpose_in(A[:], mT[:], cc)
                        Y = psum.tile([C, cfg.tile_n], f32, name="Y", tag="Y")
                        nc.tensor.matmul(
                            out=Y[:, :tn], lhsT=w1s[l][:], rhs=mT[:, :tn],
                            start=True, stop=True,
                        )
                        Ys = mlp.tile([C, cfg.tile_n], f32, name="Ys", tag="Ys")
                        nc.scalar.activation(
                            out=Ys[:, :tn], in_=Y[:, :tn], func=relu, bias=b1s[l][:]
                        )
                        Z = psum.tile([C, cfg.tile_n], f32, name="Z", tag="Y")
                        nc.tensor.matmul(
                            out=Z[:, :tn], lhsT=w2s[l][:], rhs=Ys[:, :tn],
                            start=True, stop=True,
                        )
                        Hn = mlp.tile([C, cfg.tile_n], f32, name="Hn", tag="Hn")
                        nc.scalar.activation(
                            out=Hn[:, :tn], in_=Z[:, :tn], func=relu, bias=b2s[l][:]
                        )
                        # JK fold: acc[:, tile] += h_{l+1} @ lin_W_l
                        jk = psum.tile([C, cfg.tile_n], f32, name="jk", tag="jk")
                        nc.tensor.matmul(
                            out=jk[:, :tn], lhsT=lws[l][:], rhs=Hn[:, :tn],
                            start=True, stop=True,
                        )
                        nc.vector.tensor_tensor(
                            out=acc[:, t0 : t0 + tn], in0=acc[:, t0 : t0 + tn],
                            in1=jk[:, :tn], op=add,
                        )
                        if l < NL - 1:
                            Hm = mlp.tile([P, cc_full * C], f32, name="Hm", tag="Hm")
                            transpose_out(Hn[:], Hm[:], cc)
                            nc.sync.dma_start(
                                out=shard[l][t0 : t0 + tn, :].rearrange(
                                    "(k p) f -> p k f", p=P
                                ),
                                in_=Hm[:, : cc * C].rearrange(
                                    "p (k f) -> p k f", f=C
                                ),
                            )
                    s0 += tn

                # zero this half's pad rows, then replicate to every core
                if l < NL - 1 and "aggonly" not in (debug or ""):
                    h0 = hh * cfg.hsz
                    nc.sync.dma_start(
                        out=shard[l][h0 + cfg.hreal : h0 + cfg.hsz, :],
                        in_=zeros[:npad, :C],
                    )
                    all_gather(shard[l][h0 : h0 + cfg.hsz], hf[l + 1][hh][:])
            if "aggonly" in (debug or ""):
                break

        # ---- JK bias + ReLU + store (per node tile) ----
        if "aggonly" not in (debug or ""):
            t0 = 0
            while t0 < cfg.nsh:
                tn = min(cfg.tile_n, cfg.nsh - t0)
                cc = tn // 128
                O = mlp.tile([C, cfg.tile_n], f32, name="O", tag="Hn")
                nc.scalar.activation(
                    out=O[:, :tn], in_=acc[:, t0 : t0 + tn], func=relu, bias=lb[:]
                )
                Om = mlp.tile([P, cc_full * C], f32, name="Om", tag="Hm")
                transpose_out(O[:], Om[:], cc)
                nc.sync.dma_start(
                    out=out_t.ap()[t0 : t0 + tn, :].rearrange(
                        "(k p) f -> p k f", p=P
                    ),
                    in_=Om[:, : cc * C].rearrange("p (k f) -> p k f", f=C),
                )
                t0 += tn

    nc.compile()
    return nc


# --------------------------------------------------------------------------- #
# host orchestration
# --------------------------------------------------------------------------- #
def make_in_maps(cfg: Cfg, gidx, sidx, x, weights):
    HP = cfg.ncores * cfg.hsz
    xf = None
    if cfg.x_input == "full":
        xf = np.zeros((cfg.ntot, cfg.c), np.float32)
        for c in range(cfg.ncores):
            for h in range(cfg.nhalves):
                r0 = (c * cfg.nhalves + h) * cfg.hreal
                xf[h * HP + c * cfg.hsz : h * HP + c * cfg.hsz + cfg.hreal] = x[
                    r0 : r0 + cfg.hreal
                ]
    in_maps = []
    for c in range(cfg.ncores):
        xs = np.zeros((cfg.nsh, cfg.c), np.float32)
        for h in range(cfg.nhalves):
            r0 = (c * cfg.nhalves + h) * cfg.hreal
            xs[h * cfg.hsz : h * cfg.hsz + cfg.hreal] = x[r0 : r0 + cfg.hreal]
        m = {"x_shard": xs}
        if xf is not None:
            m["x_full"] = xf
        for h in range(cfg.nhalves):
            m[f"sidx_h{h}"] = sidx[c][h]
            for w in range(cfg.nwin):
                m[f"gidx_h{h}w{w}"] = gidx[c][h][w]
        for l in range(cfg.n_layers):
            m[f"W1_{l}"] = weights[f"W1_{l}"]
            m[f"b1_{l}"] = weights[f"b1_{l}"].reshape(cfg.c, 1)
            m[f"W2_{l}"] = weights[f"W2_{l}"]
            m[f"b2_{l}"] = weights[f"b2_{l}"].reshape(cfg.c, 1)
        m["lin_W"] = weights["lin_W"]
        m["lin_b"] = weights["lin_b"].reshape(cfg.c, 1)
        in_maps.append(m)
    return in_maps


def assemble_output(cfg: Cfg, results):
    out = np.empty((cfg.n, cfg.c), np.float32)
    for c in range(cfg.ncores):
        for h in range(cfg.nhalves):
            r0 = (c * cfg.nhalves + h) * cfg.hreal
            out[r0 : r0 + cfg.hreal] = results[c]["out_shard"][
                h * cfg.hsz : h * cfg.hsz + cfg.hreal
            ]
    return out


def run_on_hw(nc, in_maps, cfg: Cfg, trace=False):
    from concourse.bass_utils import run_bass_kernel_spmd

    res = run_bass_kernel_spmd(
        nc, in_maps, core_ids=list(range(cfg.ncores)), trace=trace
    )
    return res


def kernel(**inputs) -> np.ndarray:
    x = np.asarray(inputs["x"], np.float32)
    edge_index = np.asarray(inputs["edge_index"])
    cfg = Cfg()
    assert x.shape == (cfg.n, cfg.c)
    sched, gidx, sidx, wrows, pad = preprocess(edge_index, cfg)
    nc = build_program(cfg, sched, wrows)
    in_maps = make_in_maps(cfg, gidx, sidx, x, inputs)
    res = run_on_hw(nc, in_maps, cfg)
    return assemble_output(cfg, res.results)


# revision 45
# speedup vs baseline: 1.8176x; 1.3087x over previous
"""GIN message-passing (3 layers + JumpingKnowledge cat + Linear) on 8 TRN2 NeuronCores.

Strategy (graph/data parallel, edges sharded by destination node):
  - Nodes are partitioned into 8 shards of 12544 rows, each split into 2
    half-shards of 6272 (6250 real + 22 pad) so every layer pipelines as
    gather(half1) || mlp(half0) || AllGather(half0).  Core c computes the
    aggregation + MLP for its own dst shard and gathers source features from a
    full local copy of h laid out as [half-plane][core][6272] (per-half
    AllGathers write contiguous slices; the input x itself is AllGathered
    on-device from per-core shards, so no full-size upload).
  - The segment_sum gather uses the GPSIMD dma_gather primitive (int16 indices,
    256B rows, <=1024 descriptors per call: the NRT-programmed SWDGE ring cap).
    Since int16 only addresses 32768 rows, sources are split into 4 windows of
    32768 positions; per (core, half, window) the dst nodes are sorted by
    in-degree and packed into groups of 128 with per-chunk-uniform slot counts,
    padding slots pointing at an all-zero (pad) row of h.
  - Gather index tables are uploaded de-replicated ([16, n]) and expanded to
    the ucode's 128-partition layout on-device once at kernel start.
  - Gathered tiles [128 nodes, D slots, 64ch] are segment-summed on the Vector
    engine (strided reduce over the slot axis), assembled per (half, window) in
    token (degree-sorted) order, stored to DRAM, and gathered back per node
    during the MLP phase (unique indices -> no RMW collisions).
  - MLP runs channel-major on the TensorEngine (transpose via PE identity
    matmul), biases+ReLU on the Scalar engine.  The JumpingKnowledge concat +
    final Linear is folded into the layer loop: each layer's channel-major
    activation is immediately matmul'd with its lin_W block and accumulated
    into an SBUF-resident bf16 accumulator, so layer-2 activations never touch
    DRAM and no separate JK phase re-reads the shards.
"""

import os
import sys

os.environ.setdefault("MYCRO_LOCAL_CACHE", "1")
if "/opt/trn_rl_repo" not in sys.path:
    sys.path.insert(0, "/opt/trn_rl_repo")

from contextlib import ExitStack
from dataclasses import dataclass, field

import numpy as np


# --------------------------------------------------------------------------- #
# configuration
# --------------------------------------------------------------------------- #
@dataclass
class Cfg:
    n: int = 100000          # real nodes
    e: int = 1600000
    c: int = 64              # channels (in == hid == 64)
    ncores: int = 8
    nhalves: int = 2         # half-shards per core (pipeline granularity)
    cols_max: int = 48       # gather-chunk column budget (slots per partition)
    tile_n: int = 512        # MLP node-tile width
    n_layers: int = 3
    nqueues: int = 4         # SWDGE queues; round-robin unlocks SDMA parallelism
    # NRT programs the SWDGE ring carveout at a fixed 1024 descriptors; a
    # single gather call must fit (larger calls hang in ucode await_space).
    scratch: int = 16384     # SWDGE descriptor-ring carveout (bytes/partition)
    qcols: int = 8           # gather-call column cap (128*qcols idxs <= ring)
    bn: int = 1024           # nodes per agg-gather-back call (<= ring)
    # layer-0 source: "full" uploads x replicated (free on the re-dispatch
    # path); "allgather" builds it on-device (less upload, +1 collective)
    x_input: str = "full"

    hreal: int = field(init=False)   # real nodes per half-shard
    hsz: int = field(init=False)     # rows per half-shard (multiple of 128)
    hgroups: int = field(init=False)
    nsh: int = field(init=False)     # rows per core shard
    nv: int = field(init=False)      # total virtual half-shards
    ntot: int = field(init=False)
    hp: int = field(init=False)      # rows per half-plane of hf
    window: int = field(init=False)  # gather window (never crosses a plane)
    nwin: int = field(init=False)

    def __post_init__(self):
        self.qcols = int(os.environ.get("K_QCOLS", self.qcols))
        self.bn = int(os.environ.get("K_BN", self.bn))
        self.x_input = os.environ.get("K_XINPUT", self.x_input)
        self.nhalves = int(os.environ.get("K_NH", self.nhalves))
        self.cols_max = int(os.environ.get("K_COLSMAX", self.cols_max))
        self.nqueues = int(os.environ.get("K_NQ", self.nqueues))
        assert self.n % (self.ncores * self.nhalves) == 0
        self.hreal = self.n // (self.ncores * self.nhalves)
        self.hgroups = -(-self.hreal // 128)
        self.hsz = self.hgroups * 128
        self.nsh = self.hsz * self.nhalves
        self.nv = self.ncores * self.nhalves
        self.ntot = self.hsz * self.nv
        self.hp = self.hsz * self.ncores
        # split each half-plane into equal windows of <= 32768 rows (int16)
        wsplit = -(-self.hp // 32768)
        assert self.hp % wsplit == 0
        self.window = self.hp // wsplit
        self.nwin = self.ntot // self.window
        assert self.window % self.hsz == 0, "windows must contain whole vshards"
        assert self.hsz % 16 == 0
        assert self.hsz > self.hreal, "need pad rows to host the zero rows"
        assert self.tile_n % 128 == 0
        assert self.bn % 16 == 0


# hf row of source node s: [half-plane][core][pos-in-half]
def _src_pos(cfg: Cfg, s):
    c = s // (cfg.hreal * cfg.nhalves)
    r = s % (cfg.hreal * cfg.nhalves)
    h = r // cfg.hreal
    j = r % cfg.hreal
    return h * (cfg.ncores * cfg.hsz) + c * cfg.hsz + j


@dataclass
class Sched:
    """Cross-core-uniform gather schedule, per (half, window).

    chunks[h][w] = list of (g0, [D_g ...], padc) — consecutive active groups
    packed into one SBUF tile; gather calls split at qcols columns.
    """
    chunks: list
    gmax: list
    totc: list    # [h][w] total idx columns (sum nidx/16)
    zr: list      # [w] absolute hf position of an all-zero (pad) row


# --------------------------------------------------------------------------- #
# host-side preprocessing
# --------------------------------------------------------------------------- #
def _wrap_idx(v: np.ndarray) -> np.ndarray:
    """int16 vector (len % 16 == 0) -> [16, len/16] wrapped layout.

    Logical index j lives at [j % 16, j // 16]; on-device the 16-partition
    pattern is replicated to all 128 partitions (ucode cores each read their
    own stripe).
    """
    return np.ascontiguousarray(v.reshape(-1, 16).T)  # [16, len/16]


def preprocess(edge_index: np.ndarray, cfg: Cfg):
    nc_, nh, nw, hsz = cfg.ncores, cfg.nhalves, cfg.nwin, cfg.hsz
    src = edge_index[0].astype(np.int64)
    dst = edge_index[1].astype(np.int64)

    ps = _src_pos(cfg, src)                       # hf position of each source
    w_of_src = ps // cfg.window

    c_of = dst // (cfg.hreal * nh)
    rd = dst % (cfg.hreal * nh)
    h_of = rd // cfg.hreal
    ld = rd % cfg.hreal                           # dst pos within half

    # key: (core, half, window, local-dst)
    key = (((c_of * nh + h_of) * nw) + w_of_src) * hsz + ld
    order = np.lexsort((ps, key))
    key_s = key[order]
    ps_s = ps[order]
    cnt = np.bincount(key_s, minlength=nc_ * nh * nw * hsz).reshape(
        nc_, nh, nw, hsz
    )
    base = np.zeros(nc_ * nh * nw * hsz + 1, np.int64)
    base[1:] = np.cumsum(cnt.ravel())

    # window row counts + zero rows (pad rows stay zero every layer)
    wrows = [min(cfg.window, cfg.ntot - w * cfg.window) for w in range(nw)]
    pad_pos = np.concatenate(
        [
            np.arange(v * hsz + cfg.hreal, (v + 1) * hsz)
            for v in range(cfg.nv)
        ]
    )
    zr = []
    for w in range(nw):
        cand = pad_pos[(pad_pos >= w * cfg.window) & (pad_pos < w * cfg.window + wrows[w])]
        assert cand.size, f"window {w} has no zero row"
        zr.append(int(cand[0]))

    # per-(core,half,window) degree-sorted node order
    pi = np.empty((nc_, nh, nw, hsz), np.int64)
    for c in range(nc_):
        for h in range(nh):
            for w in range(nw):
                pi[c, h, w] = np.argsort(-cnt[c, h, w], kind="stable")

    # shared schedule: per-group D = cross-core max of group max degree
    chunks = [[None] * nw for _ in range(nh)]
    gmax = [[0] * nw for _ in range(nh)]
    totc = [[0] * nw for _ in range(nh)]
    for h in range(nh):
        for w in range(nw):
            sd = -np.sort(-cnt[:, h, w, :], axis=1)
            D = sd[:, ::128].max(axis=0)           # [hgroups]
            gm = int(np.count_nonzero(D))
            ch, g0 = [], 0
            while g0 < gm:
                ds, g1 = [], g0
                while g1 < gm and sum(ds) + D[g1] <= cfg.cols_max:
                    ds.append(int(D[g1]))
                    g1 += 1
                if not ds:
                    ds = [int(D[g0])]
                    g1 = g0 + 1
                # no column padding: a short tail call is cheaper than
                # gathering pad slots
                ch.append((g0, ds, 0))
                g0 = g1
            chunks[h][w] = ch
            gmax[h][w] = gm
            totc[h][w] = sum(128 * (sum(ds) + padc) // 16 for (_, ds, padc) in ch)

    # per-core gather / scatter index arrays ([16, n] de-replicated layout)
    gidx = [[[None] * nw for _ in range(nh)] for _ in range(nc_)]
    sidx = [[None] * nh for _ in range(nc_)]
    tot_slots = 0
    for c in range(nc_):
        for h in range(nh):
            for w in range(nw):
                parts = []
                zr_loc = zr[w] - w * cfg.window
                for (g0, ds, padc) in chunks[h][w]:
                    cols = sum(ds)
                    nidx = 128 * cols
                    col2g = np.repeat(np.arange(len(ds)), ds)
                    col2d = np.concatenate([np.arange(dd) for dd in ds])
                    i = np.arange(nidx)
                    p = i & 127
                    col = i >> 7
                    r = (g0 + col2g[col]) * 128 + p
                    d = col2d[col]
                    node = pi[c, h, w][r]
                    deg = cnt[c, h, w, node]
                    k = ((c * nh + h) * nw + w) * hsz + node
                    e = base[k] + np.minimum(d, np.maximum(deg - 1, 0))
                    e = np.minimum(e, max(len(ps_s) - 1, 0))
                    v = ps_s[e] - w * cfg.window
                    v = np.where(d < deg, v, zr_loc)
                    if os.environ.get("K_FAKEIDX"):  # timing probe: tiny
                        v = v % 512                  # footprint, wrong results
                    if padc:
                        v = np.concatenate(
                            [v, np.full(128 * padc, zr_loc, np.int64)]
                        )
                    assert v.min() >= 0 and v.max() < wrows[w]
                    parts.append(_wrap_idx(v.astype(np.int16)))
                    tot_slots += nidx + 128 * padc
                gidx[c][h][w] = np.concatenate(parts, axis=1)
            # combined gather-back stream: per node, its nw window-partial
            # rows in bufw ([w*hsz + rank_w(node)]), interleaved (col%nw = w)
            assert nw * hsz <= 32768, "combined gather-back needs int16 range"
            rank = np.empty((nw, hsz), np.int64)
            for w in range(nw):
                rank[w][pi[c, h, w]] = np.arange(hsz)
            i = np.arange(nw * hsz)
            j = i >> 7
            p = i & 127
            node = (j // nw) * 128 + p
            w_ = j % nw
            v = w_ * hsz + rank[w_, node]
            sidx[c][h] = _wrap_idx(v.astype(np.int16))

    pad_frac = tot_slots / max(len(src), 1)
    return Sched(chunks, gmax, totc, zr), gidx, sidx, wrows, pad_frac


# --------------------------------------------------------------------------- #
# device program
# --------------------------------------------------------------------------- #
def _patch_queue_affine_sem_lanes():
    """Make Tile's DMASW lane assignment queue-affine.

    SWDGE completion sems are locked to the queue that first increments them
    (ucode sem_target is per-queue).  Tile cycles lanes 0..7 in scheduled
    order, which breaks once calls use queue_num 1..3.  Map queue q to lanes
    {q, q+4} so every lane only ever serves one queue.
    """
    import concourse.tile_sem_assignment as tsa
    import concourse.bass_isa as bass_isa
    import concourse.mybir as mybir

    if getattr(tsa, "_queue_affine_patched", False):
        return
    orig = tsa.TileClockTick._assign_tick
    DMAInst = tsa.DMAInst

    def _assign_tick(self, inst):
        if (
            inst.engine == mybir.EngineType.Pool
            and isinstance(inst, DMAInst)
            and not isinstance(inst, bass_isa.UserSyncedRemoteDMADescs)
            and self.swdge_sem_count == 8
        ):
            q = int(getattr(inst, "queue_num", 0) or 0) % 4
            tog = getattr(self, "_q_lane_toggle", None)
            if tog is None:
                tog = self._q_lane_toggle = {}
            t = tog.get(q, 0)
            tog[q] = t ^ 1
            self.next_sw_dma_idx = q + 4 * t
        return orig(self, inst)

    tsa.TileClockTick._assign_tick = _assign_tick
    tsa._queue_affine_patched = True


def _calibrate_swdge_cost_model():
    """Teach the Tile scheduler's cost model the HW-measured SWDGE cost.

    The stock model (994ns/call + 0.34ns/desc) underestimates dma_gather
    descriptor generation ~3x (HW-measured ~1.4us/call marginal, ~1ns/idx).
    The Pool engine is this kernel's critical path, so scheduling with the
    true ratios produces a better instruction order.  Must run before the
    first cost-model use (the rust side caches the spec extraction).
    """
    from concourse import hw_specs

    hw_specs.TRN2Spec.SWDGE_FIXED_OVERHEAD_NS = 1100
    hw_specs.TRN2Spec.SWDGE_NS_PER_DESCRIPTOR = 1.0


def build_program(cfg: Cfg, sched: Sched, wrows, debug=None, reps=1):
    import concourse.bacc as bacc
    import concourse.mybir as mybir
    import concourse.tile as tile
    from concourse.masks import make_identity

    if os.environ.get("K_CALIB", "1") == "1":
        _calibrate_swdge_cost_model()
    if cfg.nqueues > 1:
        _patch_queue_affine_sem_lanes()

    f32 = mybir.dt.float32
    bf16 = mybir.dt.bfloat16
    i16 = mybir.dt.int16
    C = cfg.c
    P = 128
    NL = cfg.n_layers
    NH = cfg.nhalves
    HP = cfg.ncores * cfg.hsz    # rows per half-plane of hf

    nc = bacc.Bacc(
        "TRN2",
        target_bir_lowering=False,
        debug=False,
        num_devices=cfg.ncores,
        num_swdge_queues=cfg.nqueues,
        dynamic_dma_scratch_size=cfg.scratch,
    )
    qrr = iter(range(10**9))  # round-robin SWDGE queue counter

    def nextq():
        return next(qrr) % cfg.nqueues

    x_shard = nc.dram_tensor("x_shard", [cfg.nsh, C], f32, kind="ExternalInput")
    x_full = (
        nc.dram_tensor("x_full", [cfg.ntot, C], f32, kind="ExternalInput")
        if cfg.x_input == "full"
        else None
    )
    gidx_t = [
        [
            nc.dram_tensor(
                f"gidx_h{h}w{w}", [16, sched.totc[h][w]], i16, kind="ExternalInput"
            )
            for w in range(cfg.nwin)
        ]
        for h in range(NH)
    ]
    sidx_t = [
        nc.dram_tensor(
            f"sidx_h{h}", [16, cfg.nwin * cfg.hsz // 16], i16, kind="ExternalInput"
        )
        for h in range(NH)
    ]
    w1_t, b1_t, w2_t, b2_t = [], [], [], []
    for l in range(NL):
        w1_t.append(nc.dram_tensor(f"W1_{l}", [C, C], f32, kind="ExternalInput"))
        b1_t.append(nc.dram_tensor(f"b1_{l}", [C, 1], f32, kind="ExternalInput"))
        w2_t.append(nc.dram_tensor(f"W2_{l}", [C, C], f32, kind="ExternalInput"))
        b2_t.append(nc.dram_tensor(f"b2_{l}", [C, 1], f32, kind="ExternalInput"))
    linw_t = nc.dram_tensor("lin_W", [NL * C, C], f32, kind="ExternalInput")
    linb_t = nc.dram_tensor("lin_b", [C, 1], f32, kind="ExternalInput")
    out_t = nc.dram_tensor("out_shard", [cfg.nsh, C], f32, kind="ExternalOutput")

    rg = [list(range(cfg.ncores))]
    add = mybir.AluOpType.add
    relu = mybir.ActivationFunctionType.Relu

    with tile.TileContext(nc) as tc, ExitStack() as ctx:
        const = ctx.enter_context(tc.tile_pool(name="const", bufs=1))
        accp = ctx.enter_context(tc.tile_pool(name="accp", bufs=1))
        idxp = ctx.enter_context(
            tc.tile_pool(name="idx", bufs=int(os.environ.get("K_IDXBUFS", 3)))
        )
        gat = ctx.enter_context(
            tc.tile_pool(name="gat", bufs=int(os.environ.get("K_GATBUFS", 3)))
        )
        asmp = ctx.enter_context(
            tc.tile_pool(name="asm", bufs=int(os.environ.get("K_ASMBUFS", 2)))
        )
        mlp = ctx.enter_context(tc.tile_pool(name="mlp", bufs=2))
        psum = ctx.enter_context(tc.tile_pool(name="psum", bufs=2, space="PSUM"))
        dram = ctx.enter_context(tc.tile_pool(name="dram", bufs=1, space="DRAM"))

        # ---- constants ----
        identity = const.tile([P, P], f32)
        make_identity(nc, identity[:])
        ZCH = 16  # zero-fill chunk (groups per DMA)
        zeros = const.tile([P, ZCH * C], f32)
        nc.vector.memset(zeros[:], 0.0)
        w1s, b1s, w2s, b2s, lws = [], [], [], [], []
        for l in range(NL):
            w1 = const.tile([C, C], f32, name=f"w1s_{l}")
            nc.sync.dma_start(out=w1[:], in_=w1_t[l].ap())
            w1s.append(w1)
            b1 = const.tile([C, 1], f32, name=f"b1s_{l}")
            nc.sync.dma_start(out=b1[:], in_=b1_t[l].ap())
            b1s.append(b1)
            w2 = const.tile([C, C], f32, name=f"w2s_{l}")
            nc.sync.dma_start(out=w2[:], in_=w2_t[l].ap())
            w2s.append(w2)
            b2 = const.tile([C, 1], f32, name=f"b2s_{l}")
            nc.sync.dma_start(out=b2[:], in_=b2_t[l].ap())
            b2s.append(b2)
            lw = const.tile([C, C], f32, name=f"lws_{l}")
            nc.sync.dma_start(out=lw[:], in_=linw_t.ap()[l * C : (l + 1) * C, :])
            lws.append(lw)
        lb = const.tile([C, 1], f32)
        nc.sync.dma_start(out=lb[:], in_=linb_t.ap())
        sis = []
        for h in range(NH):
            si = const.tile(
                [P, cfg.nwin * cfg.hsz // 16], i16, name=f"sis_{h}"
            )
            for k in range(8):
                nc.sync.dma_start(
                    out=si[16 * k : 16 * (k + 1), :], in_=sidx_t[h].ap()
                )
            sis.append(si)

        # ---- internal DRAM ----
        # ucode-layout gather indices, expanded 16 -> 128 partitions once
        gidx_rep = [
            [
                dram.tile([P, sched.totc[h][w]], i16, name=f"gidxr_{h}_{w}")
                for w in range(cfg.nwin)
            ]
            for h in range(NH)
        ]
        for h in range(NH):
            for w in range(cfg.nwin):
                for k in range(8):
                    nc.sync.dma_start(
                        out=gidx_rep[h][w][16 * k : 16 * (k + 1), :],
                        in_=gidx_t[h][w].ap(),
                    )
        # collectives can't read IO tensors: stage x_shard into internal DRAM
        xs_int = None
        if cfg.x_input != "full":
            xs_int = dram.tile([cfg.nsh, C], f32, name="xs_int")
            nc.sync.dma_start(out=xs_int[:], in_=x_shard.ap())
        # per-half partial aggregations: window w's token-ordered partials at
        # rows [w*hsz, (w+1)*hsz) (one tensor so one gather-back stream reads
        # all windows' entries for a node in a single call family)
        bufw = [
            dram.tile([cfg.nwin * cfg.hsz, C], f32, name=f"bufw_{h}")
            for h in range(NH)
        ]
        shard = [dram.tile([cfg.nsh, C], f32, name=f"shard_{l}") for l in range(NL - 1)]
        # Shared tensors allow a single writer only -> one tensor per
        # (rep, layer, half-plane); each per-half AllGather owns one.
        # hf[l][h] = AllGather of half h (layer-l output, or x for l=0).
        hf_all = [
            [
                None
                if (l == 0 and cfg.x_input == "full")
                else [
                    dram.tile(
                        [HP, C], f32, name=f"hf_{r}_{l}_{h}", addr_space="Shared"
                    )
                    for h in range(NH)
                ]
                for l in range(NL)
            ]
            for r in range(reps)
        ]

        # JumpingKnowledge accumulator: sum_l h_l @ lin_W_l, channel-major
        acc = accp.tile([C, cfg.nsh], bf16, name="jkacc")

        npad = cfg.hsz - cfg.hreal
        cc_full = cfg.tile_n // 128

        def transpose_in(src_ap, dst_ap, cc):
            """node-major [128, cc*C] -> channel-major [C, cc*128]."""
            for s in range(cc):
                pt = psum.tile([C, P], f32, name="tp", tag="tp")
                nc.tensor.transpose(
                    out=pt[:], in_=src_ap[:, s * C : (s + 1) * C], identity=identity[:]
                )
                nc.scalar.copy(out=dst_ap[:, s * P : (s + 1) * P], in_=pt[:])

        def transpose_out(src_ap, dst_ap, cc):
            """channel-major [C, cc*128] -> node-major [128, cc*C]."""
            for s in range(cc):
                pt = psum.tile([P, C], f32, name="tpo", tag="tp")
                nc.tensor.transpose(
                    out=pt[:],
                    in_=src_ap[:, s * P : (s + 1) * P],
                    identity=identity[:C, :C],
                )
                nc.scalar.copy(out=dst_ap[:, s * C : (s + 1) * C], in_=pt[:])

        def all_gather(src_ap, dst_ap):
            if "fakecc" in (debug or ""):
                # timeline-sim mode: stand in for the AllGather with a
                # local DMA of similar cost (TimelineSim can't do CC)
                nc.sync.dma_start(out=dst_ap[: cfg.hsz, :], in_=src_ap)
            else:
                nc.gpsimd.collective_compute(
                    "AllGather",
                    mybir.AluOpType.bypass,
                    replica_groups=rg,
                    ins=[src_ap],
                    outs=[dst_ap],
                )

        # (reps>1 repeats the whole 3-layer pipeline for slope-based timing;
        # results are idempotent since rep>0 re-reads the same x inputs)
        for _rep, l in ((r, ll) for r in range(reps) for ll in range(NL)):
            hf = hf_all[_rep]
            if l == 0:
                nc.vector.memset(acc[:], 0.0)
                if cfg.x_input != "full":
                    for h in range(NH):
                        all_gather(
                            xs_int[h * cfg.hsz : (h + 1) * cfg.hsz],
                            hf[0][h][:],
                        )

            wpp = HP // cfg.window    # windows per half-plane

            def win_src(w):
                r0 = (w % wpp) * cfg.window
                if (l == 0 and cfg.x_input == "full") or "gatheronly" in (
                    debug or ""
                ):
                    return x_full.ap()[w * cfg.window : (w + 1) * cfg.window, :]
                return hf[l][w // wpp][r0 : r0 + cfg.window, :]

            hcur = x_shard.ap() if l == 0 else shard[l - 1][:]

            for hh in range(NH):
                # ---- gather + segment-sum, per source window ----
                for w in range(0 if "mlponly" in (debug or "") else cfg.nwin):
                    win = win_src(w)
                    asm = asmp.tile([P, cfg.hgroups * C], f32, name="asm", tag="asm")
                    if "noreduce" in (debug or ""):
                        nc.vector.memset(asm[:], 0.0)
                    ioff = 0
                    for (g0, ds, padc) in sched.chunks[hh][w]:
                        cols = sum(ds) + padc
                        it = idxp.tile([P, cols * 8], i16, name="it", tag="it")
                        nc.sync.dma_start(
                            out=it[:], in_=gidx_rep[hh][w][:, ioff : ioff + cols * 8]
                        )
                        ioff += cols * 8
                        T = gat.tile([P, cols * C], f32, name="gt", tag="gt")
                        co = 0
                        while co < cols:
                            nco = min(cfg.qcols, cols - co)
                            nc.gpsimd.dma_gather(
                                out_ap=T[:, co * C : (co + nco) * C].rearrange(
                                    "p (k f) -> p k f", f=C
                                ),
                                in_ap=win,
                                idxs_ap=it[:, co * 8 : (co + nco) * 8],
                                num_idxs=128 * nco,
                                num_idxs_reg=128 * nco,
                                elem_size=C,
                                queue_num=nextq(),
                            )
                            co += nco
                        # segment-sum each group's slots (strided reduce)
                        if "noreduce" in (debug or ""):
                            continue
                        goff = 0
                        for gi, dd in enumerate(ds):
                            nc.vector.tensor_reduce(
                                out=asm[:, (g0 + gi) * C : (g0 + gi + 1) * C],
                                in_=T[:, goff * C : (goff + dd) * C].rearrange(
                                    "p (d f) -> p f d", f=C
                                ),
                                axis=mybir.AxisListType.X,
                                op=add,
                            )
                            goff += dd
                    if sched.gmax[hh][w] < cfg.hgroups:
                        nc.vector.memset(asm[:, sched.gmax[hh][w] * C :], 0.0)
                    # store token-ordered partials; the MLP phase gathers them
                    # back by node (no serialized scatter-add chain needed)
                    nc.sync.dma_start(
                        out=bufw[hh][w * cfg.hsz : (w + 1) * cfg.hsz].rearrange(
                            "(k p) f -> p k f", p=P
                        ),
                        in_=asm[:].rearrange("p (k f) -> p k f", f=C),
                    )

                # ---- m = h + Σ_w agg_w (combined gather-back) ; MLP ; JK ----
                NW = cfg.nwin
                s0 = 0
                while s0 < (0 if "gatheronly" in (debug or "") else cfg.hsz):
                    t0 = hh * cfg.hsz + s0     # shard-relative row
                    tn = min(cfg.tile_n, cfg.hsz - s0)
                    cc = tn // 128
                    if True:
                        A = mlp.tile([P, cc_full * C], f32, name="A", tag="A")
                        H = mlp.tile([P, cc_full * C], f32, name="H", tag="H")
                        nc.sync.dma_start(
                            out=H[:, : cc * C].rearrange("p (k f) -> p k f", f=C),
                            in_=hcur[t0 : t0 + tn, :].rearrange(
                                "(k p) f -> p k f", p=P
                            ),
                        )
                        if "mlponly" not in (debug or ""):
                            G = mlp.tile(
                                [P, cc_full * NW * C], f32, name="G", tag="G"
                            )
                            ni = NW * tn
                            io = 0
                            while io < ni:
                                nn_ = min(1024, ni - io)
                                nc.gpsimd.dma_gather(
                                    out_ap=G[
                                        :, (io // 128) * C : ((io + nn_) // 128) * C
                                    ].rearrange("p (k f) -> p k f", f=C),
                                    in_ap=bufw[hh][:],
                                    idxs_ap=sis[hh][
                                        :,
                                        (NW * s0 + io) // 16 : (NW * s0 + io + nn_)
                                        // 16,
                                    ],
                                    num_idxs=nn_,
                                    num_idxs_reg=nn_,
                                    elem_size=C,
                                    queue_num=nextq(),
                                )
                                io += nn_
                            # A[p, q, f] = Σ_w G[p, (q w) f]
                            nc.vector.tensor_reduce(
                                out=A[:, : cc * C].rearrange(
                                    "p (q f) -> p q f", f=C
                                ),
                                in_=G[:, : cc * NW * C].rearrange(
                                    "p (q w f) -> p q f w", w=NW, f=C
                                ),
                                axis=mybir.AxisListType.X,
                                op=add,
                            )
                            nc.vector.tensor_tensor(
                                out=A[:, : cc * C], in0=A[:, : cc * C],
                                in1=H[:, : cc * C], op=add,
                            )
                        else:
                            nc.vector.tensor_copy(
                                out=A[:, : cc * C], in_=H[:, : cc * C]
                            )
                        if "aggonly" in (debug or ""):
                            nc.sync.dma_start(
                                out=out_t.ap()[t0 : t0 + tn, :].rearrange(
                                    "(k p) f -> p k f", p=P
                                ),
                                in_=A[:, : cc * C].rearrange("p (k f) -> p k f", f=C),
                            )
                            s0 += tn
                            continue
                        mT = mlp.tile([C, cfg.tile_n], f32, name="mT", tag="mT")
                        transpose_in(A[:], mT[:], cc)
                        Y = psum.tile([C, cfg.tile_n], f32, name="Y", tag="Y")
                        nc.tensor.matmul(
                            out=Y[:, :tn], lhsT=w1s[l][:], rhs=mT[:, :tn],
                            start=True, stop=True,
                        )
                        Ys = mlp.tile([C, cfg.tile_n], f32, name="Ys", tag="Ys")
                        nc.scalar.activation(
                            out=Ys[:, :tn], in_=Y[:, :tn], func=relu, bias=b1s[l][:]
                        )
                        Z = psum.tile([C, cfg.tile_n], f32, name="Z", tag="Y")
                        nc.tensor.matmul(
                            out=Z[:, :tn], lhsT=w2s[l][:], rhs=Ys[:, :tn],
                            start=True, stop=True,
                        )
                        Hn = mlp.tile([C, cfg.tile_n], f32, name="Hn", tag="Hn")
                        nc.scalar.activation(
                            out=Hn[:, :tn], in_=Z[:, :tn], func=relu, bias=b2s[l][:]
                        )
                        # JK fold: acc[:, tile] += h_{l+1} @ lin_W_l
                        jk = psum.tile([C, cfg.tile_n], f32, name="jk", tag="jk")
                        nc.tensor.matmul(
                            out=jk[:, :tn], lhsT=lws[l][:], rhs=Hn[:, :tn],
                            start=True, stop=True,
                        )
                        nc.vector.tensor_tensor(
                            out=acc[:, t0 : t0 + tn], in0=acc[:, t0 : t0 + tn],
                            in1=jk[:, :tn], op=add,
                        )
                        if l < NL - 1:
                            Hm = mlp.tile([P, cc_full * C], f32, name="Hm", tag="Hm")
                            transpose_out(Hn[:], Hm[:], cc)
                            nc.sync.dma_start(
                                out=shard[l][t0 : t0 + tn, :].rearrange(
                                    "(k p) f -> p k f", p=P
                                ),
                                in_=Hm[:, : cc * C].rearrange(
                                    "p (k f) -> p k f", f=C
                                ),
                            )
                    s0 += tn

                # zero this half's pad rows, then replicate to every core
                if l < NL - 1 and not any(
                    k in (debug or "") for k in ("aggonly", "gatheronly")
                ):
                    h0 = hh * cfg.hsz
                    nc.sync.dma_start(
                        out=shard[l][h0 + cfg.hreal : h0 + cfg.hsz, :],
                        in_=zeros[:npad, :C],
                    )
                    all_gather(shard[l][h0 : h0 + cfg.hsz], hf[l + 1][hh][:])
            if "aggonly" in (debug or ""):
                break

        # ---- JK bias + ReLU + store (per node tile) ----
        if not any(k in (debug or "") for k in ("aggonly", "gatheronly")):
            t0 = 0
            while t0 < cfg.nsh:
                tn = min(cfg.tile_n, cfg.nsh - t0)
                cc = tn // 128
                O = mlp.tile([C, cfg.tile_n], f32, name="O", tag="Hn")
                nc.scalar.activation(
                    out=O[:, :tn], in_=acc[:, t0 : t0 + tn], func=relu, bias=lb[:]
                )
                Om = mlp.tile([P, cc_full * C], f32, name="Om", tag="Hm")
                transpose_out(O[:], Om[:], cc)
                nc.sync.dma_start(
                    out=out_t.ap()[t0 : t0 + tn, :].rearrange(
                        "(k p) f -> p k f", p=P
                    ),
                    in_=Om[:, : cc * C].rearrange("p (k f) -> p k f", f=C),
                )
                t0 += tn

    nc.compile()
    return nc


# --------------------------------------------------------------------------- #
# host orchestration
# --------------------------------------------------------------------------- #
def make_in_maps(cfg: Cfg, gidx, sidx, x, weights):
    HP = cfg.ncores * cfg.hsz
    xf = None
    if cfg.x_input == "full":
        xf = np.zeros((cfg.ntot, cfg.c), np.float32)
        for c in range(cfg.ncores):
            for h in range(cfg.nhalves):
                r0 = (c * cfg.nhalves + h) * cfg.hreal
                xf[h * HP + c * cfg.hsz : h * HP + c * cfg.hsz + cfg.hreal] = x[
                    r0 : r0 + cfg.hreal
                ]
    in_maps = []
    for c in range(cfg.ncores):
        xs = np.zeros((cfg.nsh, cfg.c), np.float32)
        for h in range(cfg.nhalves):
            r0 = (c * cfg.nhalves + h) * cfg.hreal
            xs[h * cfg.hsz : h * cfg.hsz + cfg.hreal] = x[r0 : r0 + cfg.hreal]
        m = {"x_shard": xs}
        if xf is not None:
            m["x_full"] = xf
        for h in range(cfg.nhalves):
            m[f"sidx_h{h}"] = sidx[c][h]
            for w in range(cfg.nwin):
                m[f"gidx_h{h}w{w}"] = gidx[c][h][w]
        for l in range(cfg.n_layers):
            m[f"W1_{l}"] = weights[f"W1_{l}"]
            m[f"b1_{l}"] = weights[f"b1_{l}"].reshape(cfg.c, 1)
            m[f"W2_{l}"] = weights[f"W2_{l}"]
            m[f"b2_{l}"] = weights[f"b2_{l}"].reshape(cfg.c, 1)
        m["lin_W"] = weights["lin_W"]
        m["lin_b"] = weights["lin_b"].reshape(cfg.c, 1)
        in_maps.append(m)
    return in_maps


def assemble_output(cfg: Cfg, results):
    out = np.empty((cfg.n, cfg.c), np.float32)
    for c in range(cfg.ncores):
        for h in range(cfg.nhalves):
            r0 = (c * cfg.nhalves + h) * cfg.hreal
            out[r0 : r0 + cfg.hreal] = results[c]["out_shard"][
                h * cfg.hsz : h * cfg.hsz + cfg.hreal
            ]
    return out


def run_on_hw(nc, in_maps, cfg: Cfg, trace=False):
    from concourse.bass_utils import run_bass_kernel_spmd

    res = run_bass_kernel_spmd(
        nc, in_maps, core_ids=list(range(cfg.ncores)), trace=trace
    )
    return res


def kernel(**inputs) -> np.ndarray:
    x = np.asarray(inputs["x"], np.float32)
    edge_index = np.asarray(inputs["edge_index"])
    cfg = Cfg()
    assert x.shape == (cfg.n, cfg.c)
    sched, gidx, sidx, wrows, pad = preprocess(edge_index, cfg)
    nc = build_program(cfg, sched, wrows)
    in_maps = make_in_maps(cfg, gidx, sidx, x, inputs)
    res = run_on_hw(nc, in_maps, cfg)
    return assemble_output(cfg, res.results)
